# revision 2
# baseline (speedup 1.0000x reference)
"""Trainium2 Bass kernel for nn_Block_28887950033544 (dense transformer block).

Shapes: x (B=2, T=2048, C=2048), H=16 heads, HS=128, MLP hidden 4C=8192.

Sharding over 8 NeuronCores:
  - attention: head-parallel (2 heads/core); qkv/attention/proj computed on
    the full 4096-token stream per core, proj produces a partial sum over
    the core's 256 input features.
  - one ReduceScatter (add) turns the 8 partial (2048 x 4096) proj outputs
    into per-core (2048 x 512) token slices.
  - MLP: token-parallel (512 tokens/core), streaming the full fc weights.

Everything on device runs in transposed activation layout (C x tokens), which
makes every matmul contraction land on the partition dim with zero on-device
transposes.  The host pre-transposes x (layout only) and re-transposes the
gathered output.

Matmuls run as float32r (full-rate fp32 mode of the PE at free-dim >= 256).
"""

import os
import sys

for _p in ("/opt/trn_rl_repo",):
    if _p not in sys.path and os.path.isdir(_p):
        sys.path.insert(0, _p)

import numpy as np

# --- problem constants (hardcoded per contract) ---
B, T, C, H = 2, 2048, 2048, 16
HS = C // H          # 128
TOK = B * T          # 4096
P = 128              # partitions
KT = C // P          # 16 k-tiles over C
NCH = TOK // 512     # 8 token chunks of 512
FF = 4 * C           # 8192
EPS = 1e-5
ISQ = float(1.0 / np.sqrt(HS))
N_CORES = 8
TPC = TOK // N_CORES   # 512 tokens per core (MLP slice)
HPC = H // N_CORES     # 2 heads per core

_BUILD_CACHE = {}
_LAST_RESULTS = {"exec_time_ns": None, "mean_exec_time_ns": None}


def _build_program(n_cores, gw1, gb1, gw2, gb2, sim_gelu=False, phases="ABCRD"):
    """Build the (SPMD, per-core identical) Bass/Tile program.

    gw1/gb1/gw2/gb2: general-path flags for nontrivial ln1_w / (ln1_b|b_qkv)
    / ln2_w / (ln2_b) handling.  The harness inputs have ln weights == 1 and
    biases == 0, so the specialized path is the one that actually runs; the
    general paths keep the kernel correct for arbitrary values.
    """
    from concourse import bacc
    import concourse.mybir as mybir
    import concourse.tile as tile

    dt = mybir.dt
    f32 = dt.float32
    f32r = dt.float32r
    AF = mybir.ActivationFunctionType
    ALU = mybir.AluOpType

    nc = bacc.Bacc("TRN2", target_bir_lowering=False, debug=False,
                   num_devices=n_cores)

    # ---- DRAM I/O ----
    xTm = nc.dram_tensor("xTm", [C, TPC], f32, kind="ExternalInput").ap()
    # weights come host-pre-tiled so every DMA is one contiguous block
    wq = nc.dram_tensor("wq", [P, KT * HPC * HS], f32,
                        kind="ExternalInput").ap()      # (p, k*f)
    wk = nc.dram_tensor("wk", [P, KT * HPC * HS], f32,
                        kind="ExternalInput").ap()
    wv = nc.dram_tensor("wv", [P, KT * HPC * HS], f32,
                        kind="ExternalInput").ap()
    wpj = nc.dram_tensor("wpj", [HPC * HS, C], f32, kind="ExternalInput").ap()
    wfc = nc.dram_tensor("wfc", [FF // P, P, KT * P], f32,
                         kind="ExternalInput").ap()     # (m, p, kt*f)
    wfc2 = nc.dram_tensor("wfc2", [8, KT, P, 8 * P], f32,
                          kind="ExternalInput").ap()    # (ch, m, p, kk*f)
    xTt = nc.dram_tensor("xTt", [NCH, KT, P, 512], f32,
                         kind="ExternalInput").ap()     # (chunk, k, p, t)
    bpjc = nc.dram_tensor("bpjc", [P, KT], f32, kind="ExternalInput").ap()
    bfcc = nc.dram_tensor("bfcc", [P, FF // P], f32, kind="ExternalInput").ap()
    bf2c = nc.dram_tensor("bf2c", [P, KT], f32, kind="ExternalInput").ap()
    ones_in = nc.dram_tensor("ones_in", [P, P], f32,
                             kind="ExternalInput").ap()
    masks_in = nc.dram_tensor("masks_in", [4 * P, 512], f32,
                              kind="ExternalInput").ap()
    if gw1:
        w1c = nc.dram_tensor("w1c", [P, KT], f32, kind="ExternalInput").ap()
        w1r = nc.dram_tensor("w1r", [1, C], f32, kind="ExternalInput").ap()
    if gb1:
        b1c = nc.dram_tensor("b1c", [P, KT], f32, kind="ExternalInput").ap()
        bqr = nc.dram_tensor("bqr", [1, HPC * HS], f32, kind="ExternalInput").ap()
        bkr = nc.dram_tensor("bkr", [1, HPC * HS], f32, kind="ExternalInput").ap()
        bvr = nc.dram_tensor("bvr", [1, HPC * HS], f32, kind="ExternalInput").ap()
    if gw2:
        w2c = nc.dram_tensor("w2c", [P, KT], f32, kind="ExternalInput").ap()
        w2r = nc.dram_tensor("w2r", [1, C], f32, kind="ExternalInput").ap()
    if gb2:
        b2c = nc.dram_tensor("b2c", [P, KT], f32, kind="ExternalInput").ap()
        bfcr = nc.dram_tensor("bfcr", [1, FF], f32, kind="ExternalInput").ap()
    out = nc.dram_tensor("out", [C, TPC], f32, kind="ExternalOutput").ap()

    def r_(ap):
        return ap.bitcast(f32r)

    # phases: either the full "ABCRD", or "small:<na>:<nd>" to scale the
    # A/D chunk loops down for load-bisection (structure stays complete).
    if phases.startswith("small"):
        _, _na, _nd = phases.split(":")
        nch_a, nchd_d = int(_na), int(_nd)
    else:
        nch_a, nchd_d = NCH, FF // 1024
    n_units = HPC * B
    nch_c = NCH
    do_rs = True
    kt_d = KT

    with tile.TileContext(nc) as tc, \
         nc.allow_low_precision(reason="float32r tiles feed the PE; all "
                                "matmul accumulation stays fp32 in PSUM"):
        with tc.tile_pool(name="dram", bufs=1, space="DRAM") as dram:
            qT_d = dram.tile([HPC * HS, TOK], f32, name="qT_d")
            kT_d = dram.tile([HPC * HS, TOK], f32, name="kT_d")
            v_d = dram.tile([TOK, HPC * HS], f32, name="v_d")
            bf16 = dt.bfloat16
            partial = [dram.tile([NCH * (C // 2), TPC], bf16, name="partial0"),
                       dram.tile([NCH * (C // 2), TPC], bf16, name="partial1")]
            if n_cores == NCH:
                rs_half = [dram.tile([C // 2, TPC], bf16, name="rs_out0"),
                           dram.tile([C // 2, TPC], bf16, name="rs_out1")]
            else:
                # test mode (n_cores=1): AllReduce is an identity copy; the
                # core's token slice is chunk 0.
                assert n_cores == 1
                rs_full = [
                    dram.tile([NCH * (C // 2), TPC], bf16, name="rs_full0"),
                    dram.tile([NCH * (C // 2), TPC], bf16, name="rs_full1")]
                rs_half = [rs_full[0][0:C // 2, :], rs_full[1][0:C // 2, :]]

            with tc.tile_pool(name="const", bufs=1) as const:
                ones_row = const.tile([1, P], f32r)       # K=1 lhsT for bcast
                nc.sync.dma_start(out=ones_row[:],
                                  in_=ones_in[0:1, :].bitcast(f32r))
                ones_col = const.tile([P, 1], f32r)       # M=1 lhsT for colsum
                nc.sync.dma_start(out=ones_col[:],
                                  in_=ones_in[:, 0:1].bitcast(f32r))
                ones_sq = const.tile([P, P], f32r)        # denominator lhsT
                nc.sync.dma_start(out=ones_sq[:],
                                  in_=ones_in[:, :].bitcast(f32r))
                eps_col = const.tile([P, 1], f32)
                nc.vector.memset(eps_col[:], EPS)
                masks = []
                for d in range(4):
                    m = const.tile([P, 512], f32, name=f"mask{d}")
                    nc.sync.dma_start(out=m[:],
                                      in_=masks_in[d * P:(d + 1) * P, :])
                    masks.append(m)
                bpjc_s = const.tile([P, KT], f32)
                nc.sync.dma_start(out=bpjc_s[:], in_=bpjc[:, :])
                bfcc_s = const.tile([P, FF // P], f32)
                nc.sync.dma_start(out=bfcc_s[:], in_=bfcc[:, :])
                bf2c_s = const.tile([P, KT], f32)
                nc.sync.dma_start(out=bf2c_s[:], in_=bf2c[:, :])
                if gw1:
                    w1c_s = const.tile([P, KT], f32)
                    nc.sync.dma_start(out=w1c_s[:], in_=w1c[:, :])
                    w1r_s = const.tile([1, C], f32r)
                    nc.sync.dma_start(out=w1r_s[:], in_=w1r[:, :].bitcast(f32r))
                if gb1:
                    b1c_s = const.tile([P, KT], f32r)
                    nc.sync.dma_start(out=b1c_s[:], in_=b1c[:, :].bitcast(f32r))
                if gw2:
                    w2c_s = const.tile([P, KT], f32)
                    nc.sync.dma_start(out=w2c_s[:], in_=w2c[:, :])
                    w2r_s = const.tile([1, C], f32r)
                    nc.sync.dma_start(out=w2r_s[:], in_=w2r[:, :].bitcast(f32r))
                if gb2:
                    b2c_s = const.tile([P, KT], f32r)
                    nc.sync.dma_start(out=b2c_s[:], in_=b2c[:, :].bitcast(f32r))

                # ======================= PHASE A: ln1 + qkv =================
                with (
                    tc.tile_pool(name="wqkv", bufs=1) as wpool,
                    tc.tile_pool(name="xchunk", bufs=1) as xpool,
                    tc.tile_pool(name="arows", bufs=2) as rows,
                    tc.tile_pool(name="astage", bufs=1) as stg,
                    tc.tile_pool(name="ps_st", bufs=1, space="PSUM") as pst,
                    tc.tile_pool(name="ps_bc", bufs=1, space="PSUM") as pbc,
                    tc.tile_pool(name="ps_qk", bufs=2, space="PSUM") as pqk,
                    tc.tile_pool(name="ps_v",
                                 bufs=(1 if (gw1 or gb1) else 2),
                                 space="PSUM") as pv,
                ):
                    wq_s = wpool.tile([P, KT * HPC * HS], f32r, tag="wq")
                    wk_s = wpool.tile([P, KT * HPC * HS], f32r, tag="wk")
                    wv_s = wpool.tile([P, KT * HPC * HS], f32r, tag="wv")
                    FW = HPC * HS  # 256
                    nc.sync.dma_start(out=wq_s[:], in_=wq[:, :].bitcast(f32r))
                    nc.sync.dma_start(out=wk_s[:], in_=wk[:, :].bitcast(f32r))
                    nc.sync.dma_start(out=wv_s[:], in_=wv[:, :].bitcast(f32r))

                    # optional on-device bias rows b' = ln1_b @ W + b_qkv
                    if gb1:
                        bprows = {}
                        for nm, ws, brow in (("q", wq_s, bqr), ("k", wk_s, bkr),
                                             ("v", wv_s, bvr)):
                            pb = pbc.tile([1, FW], f32, tag="pbias")
                            for k in range(KT):
                                nc.tensor.matmul(
                                    pb[:], r_(b1c_s[:, k:k + 1]),
                                    r_(ws[:, k * FW:(k + 1) * FW]),
                                    start=(k == 0), stop=(k == KT - 1))
                            br = rows.tile([1, FW], f32r, tag=f"bp{nm}", bufs=1)
                            bi = rows.tile([1, FW], f32, tag=f"bi{nm}", bufs=1)
                            nc.sync.dma_start(out=bi[:], in_=brow[:, :])
                            nc.vector.tensor_tensor(br[:], pb[:], bi[:], ALU.add)
                            bprows[nm] = br

                    for c in range(nch_a):
                        tok0 = c * 512
                        xk = [xpool.tile([P, 512], f32r, tag=f"x{k}", name=f"x{k}")
                              for k in range(KT)]
                        for k in range(KT):
                            nc.sync.dma_start(
                                out=xk[k][:],
                                in_=xTt[c, k, :, :].bitcast(f32r))
                        # --- stats (row form + column form) ---
                        stx = pst.tile([1, 512], f32, tag="stx")
                        stq = pst.tile([1, 512], f32, tag="stq")
                        for k in range(KT):
                            sq = stg.tile([P, 512], f32r, tag="sq", bufs=3)
                            nc.scalar.activation(sq[:], xk[k][:], AF.Square)
                            nc.tensor.matmul(stx[:], r_(ones_col[:]),
                                             r_(xk[k][:]), start=(k == 0),
                                             stop=(k == KT - 1))
                            nc.tensor.matmul(stq[:], r_(ones_col[:]),
                                             r_(sq[:]), start=(k == 0),
                                             stop=(k == KT - 1))
                        # row-form finalize: var = E[x^2] - mu^2
                        negmu = rows.tile([1, 512], f32r, tag="negmu")
                        ex2 = rows.tile([1, 512], f32, tag="ex2")
                        mu2 = rows.tile([1, 512], f32, tag="mu2")
                        var = rows.tile([1, 512], f32, tag="var")
                        std = rows.tile([1, 512], f32r, tag="std")
                        rrow = rows.tile([1, 512], f32r, tag="rrow")
                        nc.vector.tensor_scalar_mul(negmu[:], stx[:],
                                                    -1.0 / C)
                        nc.vector.tensor_scalar_mul(ex2[:], stq[:],
                                                    1.0 / C)
                        nc.vector.tensor_tensor(mu2[:], negmu[:], negmu[:],
                                                ALU.mult)
                        nc.vector.tensor_tensor(var[:], ex2[:], mu2[:],
                                                ALU.subtract)
                        nc.scalar.activation(std[:], var[:], AF.Sqrt,
                                             bias=eps_col[0:1, :])
                        nc.vector.reciprocal(rrow[:], std[:])
                        # column form of r: 4 outer-product matmuls put
                        # rrow's elements onto partitions (row -> col).
                        rcolp = pbc.tile([P, 4], f32, tag="rbp",
                                         name=f"rcolp{c}")
                        for m in range(4):
                            nc.tensor.matmul(
                                rcolp[:, m:m + 1],
                                rrow[0:1, m * P:(m + 1) * P].bitcast(f32),
                                ones_row[0:1, 0:1].bitcast(f32),
                                start=True, stop=True)
                        rcol = rows.tile([P, 4], f32, tag="rcol")
                        nc.scalar.copy(rcol[:], rcolp[:])

                        # broadcast tiles
                        if not gw1:
                            nmb = pbc.tile([P, 512], f32, tag="nmb")
                            nc.tensor.matmul(nmb[:], r_(ones_row[:]),
                                             r_(negmu[:]), start=True,
                                             stop=True)
                        rbp = pbc.tile([P, 512], f32, tag="rbp")
                        nc.tensor.matmul(rbp[:], r_(ones_row[:]), r_(rrow[:]),
                                         start=True, stop=True)
                        rb_s = stg.tile([P, 512], f32, tag="rb", bufs=2)
                        nc.scalar.copy(rb_s[:], rbp[:])

                        # xc = (x - mu) (optionally * ln1_w)
                        xc = [xpool.tile([P, 512], f32r, tag=f"xc{k}", name=f"xc{k}", bufs=2)
                              for k in range(KT)]
                        for k in range(KT):
                            if gw1:
                                nmbw = pbc.tile([P, 512], f32, tag="nmbw")
                                nc.tensor.matmul(
                                    nmbw[:],
                                    r_(w1r_s[0:1, k * P:(k + 1) * P]),
                                    r_(negmu[:]), start=True, stop=True)
                                nc.vector.scalar_tensor_tensor(
                                    xc[k][:], xk[k][:], w1c_s[:, k:k + 1],
                                    nmbw[:], ALU.mult, ALU.add)
                            else:
                                nc.vector.tensor_tensor(
                                    xc[k][:], xk[k][:], nmb[:], ALU.add)

                        # Q^T, K^T  (feat x tok), scaled by r at evict
                        for nm, ws, dst in (("q", wq_s, qT_d), ("k", wk_s, kT_d)):
                            for m in range(HPC):
                                pq = pqk.tile([P, 512], f32, tag="pqk")
                                for k in range(KT):
                                    nc.tensor.matmul(
                                        pq[:],
                                        r_(ws[:, k * FW + m * P:
                                              k * FW + (m + 1) * P]),
                                        r_(xc[k][:]),
                                        start=(k == 0),
                                        stop=(k == KT - 1 and not gb1))
                                if gb1:
                                    nc.tensor.matmul(
                                        pq[:],
                                        r_(bprows[nm][0:1, m * P:(m + 1) * P]),
                                        r_(std[:]), start=False, stop=True)
                                qs = stg.tile([P, 512], f32, tag="qkst",
                                              bufs=4)
                                nc.vector.tensor_tensor(qs[:], pq[:], rb_s[:],
                                                        ALU.mult)
                                nc.sync.dma_start(
                                    out=dst[m * P:(m + 1) * P,
                                            tok0:tok0 + 512],
                                    in_=qs[:])
                        # V (tok x feat), scaled by r (per-partition) at evict
                        for m in range(4):
                            pvt = pv.tile([P, FW], f32, tag="pv")
                            for k in range(KT):
                                nc.tensor.matmul(
                                    pvt[:],
                                    r_(xc[k][:, m * P:(m + 1) * P]),
                                    r_(wv_s[:, k * FW:(k + 1) * FW]),
                                    start=(k == 0),
                                    stop=(k == KT - 1 and not gb1))
                            if gb1:
                                nc.tensor.matmul(
                                    pvt[:], r_(std[0:1, m * P:(m + 1) * P]),
                                    r_(bprows["v"][:]), start=False, stop=True)
                            vs = stg.tile([P, FW], f32, tag="vst", bufs=4)
                            nc.scalar.activation(vs[:], pvt[:], AF.Copy,
                                                 scale=rcol[:, m:m + 1])
                            nc.sync.dma_start(
                                out=v_d[tok0 + m * P:tok0 + (m + 1) * P, :],
                                in_=vs[:])

                # ===================== PHASE B: attention ===================
                ypool_cm = tc.tile_pool(name="yT", bufs=1)
                ypool = ypool_cm.__enter__()
                yT_s = [ypool.tile([P, T], f32r, tag=f"y{u}", name=f"y{u}")
                        for u in range(HPC * B)]
                with (
                    tc.tile_pool(name="qkhb", bufs=2) as qkp,
                    tc.tile_pool(name="vhb", bufs=1) as vhp,
                    tc.tile_pool(name="expp", bufs=1) as ep,
                    tc.tile_pool(name="bstage", bufs=2) as bstg,
                    tc.tile_pool(name="ps_sc", bufs=4, space="PSUM") as psc,
                    tc.tile_pool(name="ps_dn", bufs=2, space="PSUM") as pdn,
                    tc.tile_pool(name="ps_y", bufs=2, space="PSUM") as psy,
                ):
                    NKT = T // P  # 16 key tiles per batch
                    unit_order = sorted(range(n_units), key=lambda u: (u % B, u // B))
                    for u in unit_order:
                        h, bb = u // B, u % B
                        qhb = qkp.tile([P, T], f32r, tag="qhb")
                        khb = qkp.tile([P, T], f32r, tag="khb")
                        nc.sync.dma_start(
                            out=qhb[:], in_=qT_d[h * P:(h + 1) * P,
                                               bb * T:(bb + 1) * T]
                            .bitcast(f32r))
                        nc.sync.dma_start(
                            out=khb[:], in_=kT_d[h * P:(h + 1) * P,
                                                bb * T:(bb + 1) * T]
                            .bitcast(f32r))
                        vhb = [vhp.tile([P, P], f32r, tag=f"v{k}", name=f"vhb{k}", bufs=2)
                               for k in range(NKT)]
                        for k in range(NKT):
                            nc.sync.dma_start(
                                out=vhb[k][:],
                                in_=v_d[bb * T + k * P:bb * T + (k + 1) * P,
                                        h * P:(h + 1) * P].bitcast(f32r))
                        for qc in range(T // 512):
                            nk = 4 * (qc + 1)
                            et = []
                            for kt in range(nk):
                                ps = psc.tile([P, 512], f32, tag="ps")
                                nc.tensor.matmul(
                                    ps[:], r_(khb[:, kt * P:(kt + 1) * P]),
                                    r_(qhb[:, qc * 512:(qc + 1) * 512]),
                                    start=True, stop=True)
                                e = ep.tile([P, 512], f32r, tag=f"e{kt}", name=f"e{kt}", bufs=2)
                                if kt >= 4 * qc:
                                    d = kt - 4 * qc
                                    etmp = bstg.tile([P, 512], f32, tag="ed",
                                                     bufs=3)
                                    nc.scalar.activation(etmp[:], ps[:],
                                                         AF.Exp, scale=ISQ)
                                    nc.vector.tensor_tensor(
                                        e[:], etmp[:], masks[d][:], ALU.mult)
                                else:
                                    nc.scalar.activation(e[:], ps[:], AF.Exp,
                                                         scale=ISQ)
                                et.append(e)
                            pd = pdn.tile([P, 512], f32, tag="pd")
                            for kt in range(nk):
                                nc.tensor.matmul(pd[:], r_(ones_sq[:]),
                                                 r_(et[kt][:]),
                                                 start=(kt == 0),
                                                 stop=(kt == nk - 1))
                            rc = bstg.tile([P, 512], f32, tag="rc", bufs=2)
                            nc.vector.reciprocal(rc[:], pd[:])
                            py = psy.tile([P, 512], f32, tag="py")
                            for kt in range(nk):
                                nc.tensor.matmul(py[:], r_(vhb[kt][:]),
                                                 r_(et[kt][:]),
                                                 start=(kt == 0),
                                                 stop=(kt == nk - 1))
                            nc.vector.tensor_tensor(
                                yT_s[u][:, qc * 512:(qc + 1) * 512],
                                py[:], rc[:], ALU.mult)

                # ===================== PHASE C: proj partial ================
                with (
                    tc.tile_pool(name="wpj_p", bufs=1) as wpp,
                    tc.tile_pool(name="cstage", bufs=4) as cstg,
                    tc.tile_pool(name="ps_pj", bufs=3, space="PSUM") as ppj,
                ):
                    wpj_s = [wpp.tile([P, C], f32r, tag=f"wp{k}", name=f"wp{k}")
                             for k in range(HPC)]
                    for k in range(HPC):
                        nc.sync.dma_start(out=wpj_s[k][:],
                                          in_=wpj[k * P:(k + 1) * P, :]
                                          .bitcast(f32r))
                    for m in range(KT):
                        half, mh = divmod(m, KT // 2)
                        for c in range(nch_c):
                            bb, qc = divmod(c, T // 512)
                            pp = ppj.tile([P, 512], f32, tag="pp")
                            for k in range(HPC):
                                u = k * B + bb
                                nc.tensor.matmul(
                                    pp[:], r_(wpj_s[k][:, m * P:(m + 1) * P]),
                                    r_(yT_s[u][:, qc * 512:(qc + 1) * 512]),
                                    start=(k == 0), stop=(k == HPC - 1))
                            pstg = cstg.tile([P, 512], bf16, tag="pstg",
                                             name=f"pstg{c}_{m}")
                            nc.vector.tensor_copy(pstg[:], pp[:])
                            nc.sync.dma_start(
                                out=partial[half][
                                    c * (C // 2) + mh * P:
                                    c * (C // 2) + (mh + 1) * P, :],
                                in_=pstg[:])
                ypool_cm.__exit__(None, None, None)

                # ===================== ReduceScatter ========================
                for half in range(2):
                    if n_cores == NCH and do_rs:
                        nc.gpsimd.collective_compute(
                            "ReduceScatter", ALU.add,
                            replica_groups=[list(range(n_cores))],
                            ins=[partial[half][:, :].opt()],
                            outs=[rs_half[half][:, :].opt()],
                        )
                    elif do_rs:
                        nc.gpsimd.collective_compute(
                            "AllReduce", ALU.add,
                            replica_groups=[list(range(n_cores))],
                            ins=[partial[half][:, :].opt()],
                            outs=[rs_full[half][:, :].opt()],
                        )

                # ===================== PHASE D: MLP =========================
                with (
                    tc.tile_pool(name="x2pool", bufs=1) as x2p,
                    tc.tile_pool(name="mlpst", bufs=1) as mst,
                    tc.tile_pool(name="drows", bufs=1) as drows,
                    tc.tile_pool(name="wfpool", bufs=4) as wfp,
                    tc.tile_pool(name="wgpool", bufs=3) as wgp,
                    tc.tile_pool(name="apool", bufs=1) as apool,
                    tc.tile_pool(name="dstage", bufs=1) as dstg,
                    tc.tile_pool(name="ps_st2", bufs=1, space="PSUM") as pst2,
                    tc.tile_pool(name="ps_bc2", bufs=1, space="PSUM") as pbc2,
                    tc.tile_pool(name="ps_f", bufs=2, space="PSUM") as pf,
                    tc.tile_pool(name="ps_g",
                                 bufs=(2 if (gw2 or gb2) else 3),
                                 space="PSUM") as pg,
                ):
                    db = 1 if (gw2 or gb2) else 2
                    x2t = [x2p.tile([P, TPC], f32r, tag=f"t{k}", name=f"x2t{k}")
                           for k in range(KT)]
                    x2c = [x2p.tile([P, TPC], f32r, tag=f"c{k}", name=f"x2c{k}")
                           for k in range(KT)]
                    acc = [x2p.tile([P, TPC], f32, tag=f"a{k}", name=f"acc{k}")
                           for k in range(KT)]
                    for k in range(kt_d):
                        x2r = dstg.tile([P, TPC], bf16, tag="x2r",
                                        bufs=db)
                        xmy = dstg.tile([P, TPC], f32, tag="xmy", bufs=db)
                        half, kh = divmod(k, KT // 2)
                        nc.sync.dma_start(
                            out=x2r[:],
                            in_=rs_half[half][kh * P:(kh + 1) * P, :])
                        nc.sync.dma_start(out=xmy[:],
                                          in_=xTm[k * P:(k + 1) * P, :])
                        # x2 = xT_my + rs + b_proj
                        nc.vector.scalar_tensor_tensor(
                            x2t[k][:], x2r[:], bpjc_s[:, k:k + 1], xmy[:],
                            ALU.add, ALU.add)
                        # residual accumulator init: x2 + b_fc2
                        nc.vector.tensor_scalar_add(acc[k][:], x2t[k][:],
                                                    bf2c_s[:, k:k + 1])
                    # ln2 stats (row form only)
                    st2x = pst2.tile([1, TPC], f32, tag="st2x")
                    st2q = pst2.tile([1, TPC], f32, tag="st2q")
                    for k in range(kt_d):
                        sq = dstg.tile([P, TPC], f32r, tag="sq2", bufs=db)
                        nc.scalar.activation(sq[:], x2t[k][:], AF.Square)
                        nc.tensor.matmul(st2x[:], r_(ones_col[:]),
                                         r_(x2t[k][:]), start=(k == 0),
                                         stop=(k == KT - 1))
                        nc.tensor.matmul(st2q[:], r_(ones_col[:]),
                                         r_(sq[:]), start=(k == 0),
                                         stop=(k == KT - 1))
                    negmu2 = drows.tile([1, TPC], f32r, tag="negmu2")
                    ex22 = drows.tile([1, TPC], f32, tag="ex22")
                    mu22 = drows.tile([1, TPC], f32, tag="mu22")
                    var2 = drows.tile([1, TPC], f32, tag="var2")
                    std2 = drows.tile([1, TPC], f32r, tag="std2")
                    rrow2 = drows.tile([1, TPC], f32r, tag="rrow2")
                    nc.vector.tensor_scalar_mul(negmu2[:], st2x[:],
                                                -1.0 / C)
                    nc.vector.tensor_scalar_mul(ex22[:], st2q[:],
                                                1.0 / C)
                    nc.vector.tensor_tensor(mu22[:], negmu2[:], negmu2[:],
                                            ALU.mult)
                    nc.vector.tensor_tensor(var2[:], ex22[:], mu22[:],
                                            ALU.subtract)
                    nc.scalar.activation(std2[:], var2[:], AF.Sqrt,
                                         bias=eps_col[0:1, :])
                    nc.vector.reciprocal(rrow2[:], std2[:])
                    if not gw2:
                        nmb2 = pbc2.tile([P, TPC], f32, tag="bc2", name="nmb2")
                        nc.tensor.matmul(nmb2[:], r_(ones_row[:]),
                                         r_(negmu2[:]), start=True, stop=True)
                    rb2p = pbc2.tile([P, TPC], f32, tag="bc2")
                    nc.tensor.matmul(rb2p[:], r_(ones_row[:]), r_(rrow2[:]),
                                     start=True, stop=True)
                    r2b_s = mst.tile([P, TPC], f32, tag="r2b")
                    nc.scalar.copy(r2b_s[:], rb2p[:])
                    for k in range(kt_d):
                        if gw2:
                            nmbw2 = pbc2.tile([P, TPC], f32, tag="bc2",
                                               name=f"nmbw2_{k}")
                            nc.tensor.matmul(
                                nmbw2[:], r_(w2r_s[0:1, k * P:(k + 1) * P]),
                                r_(negmu2[:]), start=True, stop=True)
                            nc.vector.scalar_tensor_tensor(
                                x2c[k][:], x2t[k][:], w2c_s[:, k:k + 1],
                                nmbw2[:], ALU.mult, ALU.add)
                        else:
                            nc.vector.tensor_tensor(x2c[k][:], x2t[k][:],
                                                    nmb2[:], ALU.add)

                    for ch in range(nchd_d):
                        f0 = ch * 1024
                        if gb2:
                            # b'fc chunk = ln2_b @ wfc[:, chunk] + b_fc[chunk]
                            # computed per m-tile from the re-tiled wfm blocks
                            bfw = [wfp.tile([P, KT * P], f32r, tag="bfw",
                                            name=f"bfw{ch}_{m}", bufs=2)
                                   for m in range(8)]
                            bfr = drows.tile([1, 1024], f32r, tag="bfr")
                            bfi = drows.tile([1, 1024], f32, tag="bfi")
                            nc.sync.dma_start(out=bfi[:],
                                              in_=bfcr[0:1, f0:f0 + 1024])
                            for m in range(8):
                                nc.sync.dma_start(
                                    out=bfw[m][:],
                                    in_=wfc[ch * 8 + m, :, :].bitcast(f32r))
                                pbm = pbc2.tile([1, P], f32, tag="pbf",
                                                name=f"pbf_{ch}_{m}")
                                for k in range(KT):
                                    nc.tensor.matmul(
                                        pbm[:],
                                        r_(b2c_s[:, k:k + 1]),
                                        r_(bfw[m][:, k * P:(k + 1) * P]),
                                        start=(k == 0), stop=(k == KT - 1))
                                nc.vector.tensor_tensor(
                                    bfr[0:1, m * P:(m + 1) * P], pbm[:],
                                    bfi[0:1, m * P:(m + 1) * P], ALU.add)
                        aT = [apool.tile([P, TPC], f32r, tag=f"aT{m}", name=f"aT{ch}_{m}", bufs=2)
                              for m in range(8)]
                        for m in range(8):
                            # all 16 C-k-tiles of wfc for this 128-wide ff
                            # m-tile, packed into one (128, 2048) tile
                            wfm = wfp.tile([P, KT * P], f32r, tag="wfm",
                                           name=f"wfm{ch}_{m}",
                                           bufs=(2 if (gw2 or gb2) else 3))
                            nc.sync.dma_start(
                                out=wfm[:],
                                in_=wfc[ch * 8 + m, :, :].bitcast(f32r))
                            pft = pf.tile([P, TPC], f32, tag="pf")
                            for k in range(KT):
                                nc.tensor.matmul(
                                    pft[:], r_(wfm[:, k * P:(k + 1) * P]),
                                    r_(x2c[k][:]), start=(k == 0),
                                    stop=(k == KT - 1 and not gb2))
                            if gb2:
                                nc.tensor.matmul(
                                    pft[:], r_(bfr[0:1, m * P:(m + 1) * P]),
                                    r_(std2[:]), start=False, stop=True)
                            tmp = dstg.tile([P, TPC], f32, tag="tmp", bufs=db + 1)
                            nc.vector.tensor_tensor(tmp[:], pft[:], r2b_s[:],
                                                    ALU.mult)
                            gbias = (0.0 if gb2
                                     else bfcc_s[:, ch * 8 + m:ch * 8 + m + 1])
                            if not sim_gelu:
                                nc.scalar.activation(aT[m][:], tmp[:], AF.Gelu,
                                                     bias=gbias)
                            else:
                                # CoreSim has no Gelu -- tanh-approx expansion
                                xg = dstg.tile([P, TPC], f32, tag="xg", bufs=db)
                                nc.scalar.activation(xg[:], tmp[:],
                                                     AF.Identity, bias=gbias)
                                sqg = dstg.tile([P, TPC], f32, tag="sq2",
                                                bufs=db)
                                nc.scalar.activation(sqg[:], xg[:], AF.Square)
                                nc.vector.tensor_scalar(sqg[:], sqg[:],
                                                        0.044715, 1.0,
                                                        ALU.mult, ALU.add)
                                nc.vector.tensor_tensor(sqg[:], sqg[:], xg[:],
                                                        ALU.mult)
                                nc.scalar.activation(sqg[:], sqg[:], AF.Tanh,
                                                     scale=0.7978845608028654)
                                nc.vector.tensor_scalar(sqg[:], sqg[:], 1.0,
                                                        0.5, ALU.add, ALU.mult)
                                nc.vector.tensor_tensor(aT[m][:], sqg[:],
                                                        xg[:], ALU.mult)
                        for m in range(KT):
                            # 8 ff-k-tiles of wfc2 for this 128-wide C m-tile
                            wgm = wgp.tile([P, 8 * P], f32r, tag="wgm",
                                           name=f"wgm{ch}_{m}",
                                           bufs=(2 if (gw2 or gb2) else 3))
                            nc.sync.dma_start(
                                out=wgm[:],
                                in_=wfc2[ch, m, :, :].bitcast(f32r))
                            pgt = pg.tile([P, TPC], f32, tag="pg")
                            for kk in range(8):
                                nc.tensor.matmul(
                                    pgt[:], r_(wgm[:, kk * P:(kk + 1) * P]),
                                    r_(aT[kk][:]), start=(kk == 0),
                                    stop=(kk == 7))
                            nc.vector.tensor_tensor(acc[m][:], pgt[:],
                                                    acc[m][:], ALU.add)
                    for m in range(KT):
                        nc.sync.dma_start(out=out[m * P:(m + 1) * P, :],
                                          in_=acc[m][:])

    nc.compile()
    return nc


def _get_program(n_cores, flags, sim_gelu=False):
    key = (n_cores, flags, sim_gelu)
    if key not in _BUILD_CACHE:
        _BUILD_CACHE[key] = _build_program(n_cores, *flags,
                                           sim_gelu=sim_gelu)
    return _BUILD_CACHE[key]


def _colmajor(v, kt):
    """(kt*128,) vector -> (128, kt) column-tile layout."""
    return np.ascontiguousarray(v.reshape(kt, P).T)


def make_in_maps(x, ln1_w, ln1_b, w_qkv, b_qkv, w_proj, b_proj,
                 ln2_w, ln2_b, w_fc, b_fc, w_fc2, b_fc2, n_cores=N_CORES):
    """Host-side sharding: slicing / transpose / reshape only."""
    f = np.float32
    x2d = np.ascontiguousarray(x.reshape(TOK, C), dtype=f)
    xT = np.ascontiguousarray(x2d.T)
    flags = (
        not np.all(ln1_w == 1.0),
        not (np.all(ln1_b == 0.0) and np.all(b_qkv == 0.0)),
        not np.all(ln2_w == 1.0),
        not np.all(ln2_b == 0.0),
    )
    gw1, gb1, gw2, gb2 = flags
    w_qkv = np.asarray(w_qkv, f)
    # causal mask tiles: mask[d][kk, qq] = 1 if qq - kk - 128*d >= 0
    _kk = np.arange(P)[:, None]
    _qq = np.arange(512)[None, :]
    _masks = np.concatenate(
        [(_qq - _kk - 128 * d >= 0).astype(f) for d in range(4)], axis=0)
    # pre-tiled weight layouts (pure host-side reshapes/transposes):
    #   wfc  (C, FF)  -> (FF/P m, P p, KT*P kf): block m holds all C-k-tiles
    #   wfc2 (FF, C)  -> (8 ch, KT m, P p, 8*P kkf)
    #   xT   (C, TOK) -> (NCH c, KT k, P p, 512 t)
    wfc_t = np.ascontiguousarray(
        np.asarray(w_fc, f).reshape(KT, P, FF // P, P)
        .transpose(2, 1, 0, 3).reshape(FF // P, P, KT * P))
    wfc2_t = np.ascontiguousarray(
        np.asarray(w_fc2, f).reshape(8, 8, P, KT, P)
        .transpose(0, 3, 2, 1, 4).reshape(8, KT, P, 8 * P))
    xT_t = np.ascontiguousarray(
        xT.reshape(KT, P, NCH, 512).transpose(2, 0, 1, 3))
    shared = {
        "xTt": xT_t,
        "ones_in": np.ones((P, P), f),
        "masks_in": _masks,
        "wfc": wfc_t,
        "wfc2": wfc2_t,
        "bpjc": _colmajor(np.asarray(b_proj, f), KT),
        "bfcc": _colmajor(np.asarray(b_fc, f), FF // P),
        "bf2c": _colmajor(np.asarray(b_fc2, f), KT),
    }
    if gw1:
        shared["w1c"] = _colmajor(np.asarray(ln1_w, f), KT)
        shared["w1r"] = np.asarray(ln1_w, f)[None, :]
    if gb1:
        shared["b1c"] = _colmajor(np.asarray(ln1_b, f), KT)
    if gw2:
        shared["w2c"] = _colmajor(np.asarray(ln2_w, f), KT)
        shared["w2r"] = np.asarray(ln2_w, f)[None, :]
    if gb2:
        shared["b2c"] = _colmajor(np.asarray(ln2_b, f), KT)
        shared["bfcr"] = np.asarray(b_fc, f)[None, :]
    in_maps = []
    FW = HPC * HS
    for c in range(n_cores):
        m = dict(shared)
        m["xTm"] = np.ascontiguousarray(xT[:, c * TPC:(c + 1) * TPC])
        def _kpf(w):  # (C, FW) -> (P p, KT*FW kf)
            return np.ascontiguousarray(
                w.reshape(KT, P, FW).transpose(1, 0, 2).reshape(P, KT * FW))
        m["wq"] = _kpf(w_qkv[:, c * FW:(c + 1) * FW])
        m["wk"] = _kpf(w_qkv[:, C + c * FW:C + (c + 1) * FW])
        m["wv"] = _kpf(w_qkv[:, 2 * C + c * FW:2 * C + (c + 1) * FW])
        m["wpj"] = np.ascontiguousarray(
            np.asarray(w_proj, f)[c * FW:(c + 1) * FW, :])
        if gb1:
            bq = np.asarray(b_qkv, f)
            m["bqr"] = np.ascontiguousarray(bq[None, c * FW:(c + 1) * FW])
            m["bkr"] = np.ascontiguousarray(
                bq[None, C + c * FW:C + (c + 1) * FW])
            m["bvr"] = np.ascontiguousarray(
                bq[None, 2 * C + c * FW:2 * C + (c + 1) * FW])
        in_maps.append(m)
    return in_maps, flags


def kernel(**inputs):
    from concourse.bass_utils import run_bass_kernel_spmd

    in_maps, flags = make_in_maps(**inputs)
    nc = _get_program(N_CORES, flags)

    trace = os.environ.get("KERNEL_TRACE", "0") == "1"
    kw = {}
    if trace:
        kw = dict(trace=True)
    try:
        res = run_bass_kernel_spmd(nc, in_maps, list(range(N_CORES)), **kw)
    except Exception as e:
        if not trace:
            raise
        _LAST_RESULTS["trace_error"] = repr(e)
        res = run_bass_kernel_spmd(nc, in_maps, list(range(N_CORES)))
    _LAST_RESULTS["exec_time_ns"] = res.exec_time_ns
    _LAST_RESULTS["mean_exec_time_ns"] = res.mean_exec_time_ns
    _LAST_RESULTS["results"] = res
    outT = np.concatenate([res.results[i]["out"] for i in range(N_CORES)],
                          axis=1)
    return np.ascontiguousarray(outT.T).reshape(B, T, C).astype(np.float32)



# revision 9
# speedup vs baseline: 1.2748x; 1.2748x over previous
"""Trainium2 Bass kernel for nn_Block_28887950033544 (dense transformer block).

Shapes: x (B=2, T=2048, C=2048), H=16 heads, HS=128, MLP hidden 4C=8192.

v2 sharding over 8 NeuronCores:
  - attention: head-parallel (2 heads/core); qkv computed on the full
    4096-token stream per core for the core's heads; q/k/v stay SBUF-resident
    in bf16 (no DRAM roundtrip).
  - after attention, two 1MB AllToAll collectives (one per local-head slot)
    redistribute y from head-sharded to token-sharded layout.
  - proj + MLP: token-parallel (512 tokens/core).

All heavy matmuls run in bf16 (inputs are bf16; PSUM accumulation is fp32).
LayerNorm centering is folded into the matmuls as rank-1 corrections
(colsum(W) x mu), so the projections never wait on the stats.
ln1_w/ln2_w are folded into the weights host-side; ln biases fold into
host-precomputed bias columns/rows.

Everything on device runs in transposed activation layout (C x tokens).
"""

import os
import sys

for _p in ("/opt/trn_rl_repo",):
    if _p not in sys.path and os.path.isdir(_p):
        sys.path.insert(0, _p)

import numpy as np

# --- problem constants (hardcoded per contract) ---
B, T, C, H = 2, 2048, 2048, 16
HS = C // H          # 128
TOK = B * T          # 4096
P = 128              # partitions
KT = C // P          # 16 k-tiles over C
NCH = TOK // 512     # 8 token chunks of 512
FF = 4 * C           # 8192
EPS = 1e-5
ISQ = float(1.0 / np.sqrt(HS))
N_CORES = 8
TPC = TOK // N_CORES   # 512 tokens per core (proj/MLP slice)
HPC = H // N_CORES     # 2 heads per core
FW = HPC * HS          # 256

_BUILD_CACHE = {}
_LAST_RESULTS = {"exec_time_ns": None, "mean_exec_time_ns": None}


def _build_program(n_cores, gb1):
    """Build the (SPMD, per-core identical) Bass/Tile program.

    gb1: general-path flag for a nontrivial fused qkv bias
    (ln1_b @ W + b_qkv != 0).  The harness inputs have zero biases, so the
    specialized path is the one that actually runs.
    """
    from concourse import bacc
    import concourse.mybir as mybir
    import concourse.tile as tile

    dt = mybir.dt
    f32 = dt.float32
    f32r = dt.float32r
    bf16 = dt.bfloat16
    AF = mybir.ActivationFunctionType
    ALU = mybir.AluOpType

    nc = bacc.Bacc("TRN2", target_bir_lowering=False, debug=False,
                   num_devices=n_cores)

    # ---- DRAM I/O ----
    # full token stream, transposed, chunk-major: [NCH, P, KT*512] bf16
    xTt = nc.dram_tensor("xTt", [NCH, P, KT * 512], bf16,
                         kind="ExternalInput").ap()
    xTm = nc.dram_tensor("xTm", [C, TPC], f32, kind="ExternalInput").ap()
    wq = nc.dram_tensor("wq", [P, KT * FW], bf16, kind="ExternalInput").ap()
    wk = nc.dram_tensor("wk", [P, KT * FW], bf16, kind="ExternalInput").ap()
    wv = nc.dram_tensor("wv", [P, KT * FW], bf16, kind="ExternalInput").ap()
    csqkv = nc.dram_tensor("csqkv", [1, 3 * FW], bf16,
                           kind="ExternalInput").ap()
    wpj = nc.dram_tensor("wpj", [2 * NCH, P, C], bf16,
                         kind="ExternalInput").ap()   # [hl*8+j] head (2j+hl)
    wfc = nc.dram_tensor("wfc", [FF // P, P, KT * P], bf16,
                         kind="ExternalInput").ap()
    csfc = nc.dram_tensor("csfc", [1, FF], bf16, kind="ExternalInput").ap()
    wfc2 = nc.dram_tensor("wfc2", [NCH, KT, P, NCH * P], bf16,
                          kind="ExternalInput").ap()
    bpjc = nc.dram_tensor("bpjc", [P, KT], f32, kind="ExternalInput").ap()
    bfcc = nc.dram_tensor("bfcc", [P, FF // P], f32, kind="ExternalInput").ap()
    bf2c = nc.dram_tensor("bf2c", [P, KT], f32, kind="ExternalInput").ap()
    ones_f = nc.dram_tensor("ones_f", [P, P], f32, kind="ExternalInput").ap()
    ones_b = nc.dram_tensor("ones_b", [P, P], bf16, kind="ExternalInput").ap()
    masks_in = nc.dram_tensor("masks_in", [2 * P, 1024], bf16,
                              kind="ExternalInput").ap()
    if gb1:
        bqr = nc.dram_tensor("bqr", [1, FW], bf16, kind="ExternalInput").ap()
        bkr = nc.dram_tensor("bkr", [1, FW], bf16, kind="ExternalInput").ap()
        bvr = nc.dram_tensor("bvr", [1, FW], bf16, kind="ExternalInput").ap()
    out = nc.dram_tensor("out", [C, TPC], f32, kind="ExternalOutput").ap()

    def rr(ap):
        return ap.bitcast(f32r)

    with tile.TileContext(nc) as tc, \
         nc.allow_low_precision(reason="bf16 matmul inputs; all matmul "
                                "accumulation stays fp32 in PSUM"):
        with tc.tile_pool(name="dram", bufs=1, space="DRAM") as dram:
            a2a_in = [dram.tile([NCH * P, 512], bf16, name=f"a2a_in{hl}")
                      for hl in range(2)]
            a2a_out = [dram.tile([NCH * P, 512], bf16, name=f"a2a_out{hl}")
                       for hl in range(2)]

            with tc.tile_pool(name="const", bufs=1) as const:
                ones_row = const.tile([1, P], f32r)     # f32r bcast lhsT
                nc.sync.dma_start(out=ones_row[:],
                                  in_=ones_f[0:1, :].bitcast(f32r))
                ones_colb = const.tile([P, 1], bf16)    # stats lhsT
                nc.sync.dma_start(out=ones_colb[:], in_=ones_b[:, 0:1])
                ones_sqb = const.tile([P, P], bf16)     # denom lhsT
                nc.sync.dma_start(out=ones_sqb[:], in_=ones_b[:, :])
                eps_col = const.tile([P, 1], f32)
                nc.vector.memset(eps_col[:], EPS)
                masks = []
                for d in range(2):
                    m = const.tile([P, 1024], bf16, name=f"mask{d}")
                    nc.sync.dma_start(out=m[:],
                                      in_=masks_in[d * P:(d + 1) * P, :])
                    masks.append(m)
                bpjc_s = const.tile([P, KT], f32)
                nc.sync.dma_start(out=bpjc_s[:], in_=bpjc[:, :])
                bfcc_s = const.tile([P, FF // P], f32)
                nc.sync.dma_start(out=bfcc_s[:], in_=bfcc[:, :])
                bf2c_s = const.tile([P, KT], f32)
                nc.sync.dma_start(out=bf2c_s[:], in_=bf2c[:, :])
                csq_s = const.tile([1, 3 * FW], bf16)
                nc.sync.dma_start(out=csq_s[:], in_=csqkv[:, :])
                if gb1:
                    b_rows = {}
                    for nm, src in (("q", bqr), ("k", bkr), ("v", bvr)):
                        t = const.tile([1, FW], bf16, name=f"brow_{nm}")
                        nc.sync.dma_start(out=t[:], in_=src[:, :])
                        b_rows[nm] = t

                # persistent bf16 activations (SBUF-resident across phases)
                with tc.tile_pool(name="qkv_sb", bufs=1) as qkvp:
                    qT_sb = [qkvp.tile([P, TOK], bf16, name=f"qT{m}")
                             for m in range(HPC)]
                    kT_sb = [qkvp.tile([P, TOK], bf16, name=f"kT{m}")
                            for m in range(HPC)]
                    v_sb = [qkvp.tile([P, FW], bf16, name=f"v{i}")
                            for i in range(TOK // P)]

                    # ================= PHASE A: ln1 + qkv =================
                    with (
                        tc.tile_pool(name="wqkv", bufs=1) as wpool,
                        tc.tile_pool(name="xchunk", bufs=2) as xpool,
                        tc.tile_pool(name="arows", bufs=2) as rows,
                        tc.tile_pool(name="astage", bufs=1) as stg,
                        tc.tile_pool(name="ps_st", bufs=1, space="PSUM") as pst,
                        tc.tile_pool(name="ps_bc", bufs=1, space="PSUM") as pbc,
                        tc.tile_pool(name="ps_qk", bufs=3, space="PSUM") as pqk,
                        tc.tile_pool(name="ps_v", bufs=2, space="PSUM") as pv,
                    ):
                        wq_s = wpool.tile([P, KT * FW], bf16, tag="wq")
                        wk_s = wpool.tile([P, KT * FW], bf16, tag="wk")
                        wv_s = wpool.tile([P, KT * FW], bf16, tag="wv")
                        nc.sync.dma_start(out=wq_s[:], in_=wq[:, :])
                        nc.sync.dma_start(out=wk_s[:], in_=wk[:, :])
                        nc.sync.dma_start(out=wv_s[:], in_=wv[:, :])

                        for c in range(NCH):
                            tok0 = c * 512
                            xb = xpool.tile([P, KT * 512], bf16, tag="xb",
                                            name=f"xb{c}")
                            nc.sync.dma_start(out=xb[:], in_=xTt[c, :, :])
                            xk = [xb[:, k * 512:(k + 1) * 512]
                                  for k in range(KT)]
                            # --- stats ---
                            stx = pst.tile([1, 512], f32, tag="stx")
                            stq = pst.tile([1, 512], f32, tag="stq")
                            for k in range(KT):
                                sq = stg.tile([P, 512], bf16, tag="sq",
                                              bufs=3)
                                nc.vector.tensor_tensor(sq[:], xk[k], xk[k],
                                                        ALU.mult)
                                nc.tensor.matmul(stx[:], ones_colb[:], xk[k],
                                                 start=(k == 0),
                                                 stop=(k == KT - 1))
                                nc.tensor.matmul(stq[:], ones_colb[:], sq[:],
                                                 start=(k == 0),
                                                 stop=(k == KT - 1))
                            negmu = rows.tile([1, 512], f32r, tag="negmu")
                            negmuh = rows.tile([1, 512], bf16, tag="negmuh")
                            ex2 = rows.tile([1, 512], f32, tag="ex2")
                            mu2 = rows.tile([1, 512], f32, tag="mu2")
                            var = rows.tile([1, 512], f32, tag="var")
                            std = rows.tile([1, 512], f32r, tag="std")
                            rrow = rows.tile([1, 512], f32r, tag="rrow")
                            if gb1:
                                stdh = rows.tile([1, 512], bf16, tag="stdh")
                            nc.vector.tensor_scalar_mul(negmu[:], stx[:],
                                                        -1.0 / C)
                            nc.vector.tensor_copy(negmuh[:], negmu[:])
                            nc.vector.tensor_scalar_mul(ex2[:], stq[:],
                                                        1.0 / C)
                            nc.vector.tensor_tensor(mu2[:], negmu[:],
                                                    negmu[:], ALU.mult)
                            nc.vector.tensor_tensor(var[:], ex2[:], mu2[:],
                                                    ALU.subtract)
                            nc.scalar.activation(std[:], var[:], AF.Sqrt,
                                                 bias=eps_col[0:1, :])
                            nc.vector.reciprocal(rrow[:], std[:])
                            if gb1:
                                nc.vector.tensor_copy(stdh[:], std[:])
                            # r as column form (4 outer products) + bcast
                            rcolp = pbc.tile([P, 4], f32, tag="rbp",
                                             name=f"rcolp{c}")
                            for m in range(4):
                                nc.tensor.matmul(
                                    rcolp[:, m:m + 1],
                                    rrow[0:1, m * P:(m + 1) * P].bitcast(f32),
                                    ones_row[0:1, 0:1].bitcast(f32),
                                    start=True, stop=True)
                            rcol = rows.tile([P, 4], f32, tag="rcol")
                            nc.scalar.copy(rcol[:], rcolp[:])
                            rbp = pbc.tile([P, 512], f32, tag="rbp")
                            nc.tensor.matmul(rbp[:], rr(ones_row[:]),
                                             rr(rrow[:]), start=True,
                                             stop=True)
                            rb_s = stg.tile([P, 512], f32, tag="rb", bufs=2)
                            nc.scalar.copy(rb_s[:], rbp[:])

                            # Q^T, K^T (feat x tok) on raw x + rank-1 fix
                            for qk_i, (ws, dst) in enumerate(
                                    ((wq_s, qT_sb), (wk_s, kT_sb))):
                                for m in range(HPC):
                                    pq = pqk.tile([P, 512], f32, tag="pqk")
                                    for k in range(KT):
                                        nc.tensor.matmul(
                                            pq[:],
                                            ws[:, k * FW + m * P:
                                               k * FW + (m + 1) * P],
                                            xk[k],
                                            start=(k == 0), stop=False)
                                    cs0 = qk_i * FW + m * P
                                    nc.tensor.matmul(
                                        pq[:], csq_s[0:1, cs0:cs0 + P],
                                        negmuh[:], start=False,
                                        stop=(not gb1))
                                    if gb1:
                                        br = b_rows["q" if qk_i == 0 else "k"]
                                        nc.tensor.matmul(
                                            pq[:], br[0:1, m * P:(m + 1) * P],
                                            stdh[:], start=False, stop=True)
                                    nc.vector.tensor_tensor(
                                        dst[m][:, tok0:tok0 + 512],
                                        pq[:], rb_s[:], ALU.mult)
                            # V (tok x feat) on raw x + rank-1 fix
                            for mt in range(4):
                                pvt = pv.tile([P, FW], f32, tag="pv")
                                for k in range(KT):
                                    nc.tensor.matmul(
                                        pvt[:],
                                        xk[k][:, mt * P:(mt + 1) * P],
                                        wv_s[:, k * FW:(k + 1) * FW],
                                        start=(k == 0), stop=False)
                                nc.tensor.matmul(
                                    pvt[:],
                                    negmuh[0:1, mt * P:(mt + 1) * P],
                                    csq_s[0:1, 2 * FW:3 * FW],
                                    start=False, stop=(not gb1))
                                if gb1:
                                    nc.tensor.matmul(
                                        pvt[:],
                                        stdh[0:1, mt * P:(mt + 1) * P],
                                        b_rows["v"][:],
                                        start=False, stop=True)
                                nc.scalar.activation(
                                    v_sb[c * 4 + mt][:], pvt[:], AF.Copy,
                                    scale=rcol[:, mt:mt + 1])

                    # ================= PHASE B: attention =================
                    with (
                        tc.tile_pool(name="expp", bufs=3) as ep,
                        tc.tile_pool(name="bstage", bufs=2) as bstg,
                        tc.tile_pool(name="ystage", bufs=2) as ystg,
                        tc.tile_pool(name="ps_sc", bufs=2, space="PSUM") as psc,
                        tc.tile_pool(name="ps_dn", bufs=2, space="PSUM") as pdn,
                        tc.tile_pool(name="ps_y", bufs=2, space="PSUM") as psy,
                    ):
                        for u, (hl, bb) in enumerate(
                                ((0, 0), (0, 1), (1, 0), (1, 1))):
                            qhb = qT_sb[hl][:, bb * T:(bb + 1) * T]
                            khb = kT_sb[hl][:, bb * T:(bb + 1) * T]
                            yT = ystg.tile([P, T], bf16, tag="yT",
                                           name=f"yT{u}")
                            for qc in range(T // 512):
                                nk = 4 * (qc + 1)
                                ebigs = []
                                for g in range(nk // 2):
                                    ps = psc.tile([P, 1024], f32, tag="sc")
                                    for i in range(2):
                                        kt = 2 * g + i
                                        nc.tensor.matmul(
                                            ps[:, i * 512:(i + 1) * 512],
                                            khb[:, kt * P:(kt + 1) * P],
                                            qhb[:, qc * 512:(qc + 1) * 512],
                                            start=True, stop=True)
                                    e = ep.tile([P, 1024], bf16, tag="e",
                                                name=f"e{g}", bufs=4)
                                    if 2 * g >= 4 * qc:
                                        etmp = bstg.tile([P, 1024], bf16,
                                                         tag="ed", bufs=2)
                                        nc.scalar.activation(etmp[:], ps[:],
                                                             AF.Exp,
                                                             scale=ISQ)
                                        nc.vector.tensor_tensor(
                                            e[:], etmp[:],
                                            masks[(2 * g - 4 * qc) // 2][:],
                                            ALU.mult)
                                    else:
                                        nc.scalar.activation(e[:], ps[:],
                                                             AF.Exp,
                                                             scale=ISQ)
                                    ebigs.append(e)
                                pd = pdn.tile([P, 512], f32, tag="pd")
                                py = psy.tile([P, 512], f32, tag="py")
                                for kt in range(nk):
                                    sl = ebigs[kt // 2][:, (kt % 2) * 512:
                                                        (kt % 2) * 512 + 512]
                                    nc.tensor.matmul(pd[:], ones_sqb[:], sl,
                                                     start=(kt == 0),
                                                     stop=(kt == nk - 1))
                                    vt = v_sb[bb * 16 + kt]
                                    nc.tensor.matmul(
                                        py[:], vt[:, hl * P:(hl + 1) * P], sl,
                                        start=(kt == 0), stop=(kt == nk - 1))
                                rc = bstg.tile([P, 512], f32, tag="rc",
                                               bufs=2)
                                nc.vector.reciprocal(rc[:], pd[:])
                                nc.vector.tensor_tensor(
                                    yT[:, qc * 512:(qc + 1) * 512],
                                    py[:], rc[:], ALU.mult)
                            for j in range(4):
                                nc.sync.dma_start(
                                    out=a2a_in[hl][(bb * 4 + j) * P:
                                                   (bb * 4 + j + 1) * P, :],
                                    in_=yT[:, j * 512:(j + 1) * 512])
                            if bb == 1:
                                if n_cores > 1:
                                    nc.gpsimd.collective_compute(
                                        "AllToAll", ALU.bypass,
                                        replica_groups=[list(range(n_cores))],
                                        ins=[a2a_in[hl][:, :].opt()],
                                        outs=[a2a_out[hl][:, :].opt()],
                                    )
                                else:
                                    nc.sync.dma_start(out=a2a_out[hl][:, :],
                                                      in_=a2a_in[hl][:, :])

                # =============== PHASE C: proj (own tokens) ===============
                with (
                    tc.tile_pool(name="x2pool", bufs=1) as x2p,
                    tc.tile_pool(name="drows", bufs=1) as drows,
                ):
                    acc = [x2p.tile([P, TPC], f32, name=f"acc{m}")
                           for m in range(KT)]
                    x2b = [x2p.tile([P, TPC], bf16, name=f"x2b{m}")
                           for m in range(KT)]
                    negmu2 = drows.tile([1, TPC], f32r, tag="negmu2")
                    negmu2h = drows.tile([1, TPC], bf16, tag="negmu2h")
                    r2b_s = drows.tile([P, TPC], f32, tag="r2b")
                    with (
                        tc.tile_pool(name="wpj_p", bufs=1) as wpp,
                        tc.tile_pool(name="ygp", bufs=1) as ygp,
                        tc.tile_pool(name="cstage", bufs=2) as cstg,
                        tc.tile_pool(name="ps_pj", bufs=3, space="PSUM") as ppj,
                        tc.tile_pool(name="ps_st2", bufs=1,
                                     space="PSUM") as pst2,
                        tc.tile_pool(name="ps_bc2", bufs=1,
                                     space="PSUM") as pbc2,
                    ):
                        st2x = pst2.tile([1, TPC], f32, tag="st2x")
                        st2q = pst2.tile([1, TPC], f32, tag="st2q")
                        wpj_s = {}
                        yg = {}
                        for hl in range(2):
                            for j in range(NCH):
                                w = wpp.tile([P, C], bf16, tag=f"wpj{hl}_{j}",
                                             name=f"wpj{hl}_{j}")
                                nc.sync.dma_start(out=w[:],
                                                  in_=wpj[hl * NCH + j, :, :])
                                wpj_s[(hl, j)] = w
                                y = ygp.tile([P, 512], bf16,
                                             tag=f"yg{hl}_{j}",
                                             name=f"yg{hl}_{j}")
                                nc.sync.dma_start(
                                    out=y[:],
                                    in_=a2a_out[hl][j * P:(j + 1) * P, :])
                                yg[(hl, j)] = y
                        for hl in range(2):
                            for m in range(KT):
                                pp = ppj.tile([P, TPC], f32, tag="pp")
                                for j in range(NCH):
                                    nc.tensor.matmul(
                                        pp[:],
                                        wpj_s[(hl, j)][:, m * P:(m + 1) * P],
                                        yg[(hl, j)][:],
                                        start=(j == 0), stop=(j == NCH - 1))
                                if hl == 0:
                                    xmy = cstg.tile([P, TPC], f32, tag="xmy",
                                                    bufs=3)
                                    nc.sync.dma_start(
                                        out=xmy[:],
                                        in_=xTm[m * P:(m + 1) * P, :])
                                    nc.vector.scalar_tensor_tensor(
                                        acc[m][:], pp[:], bpjc_s[:, m:m + 1],
                                        xmy[:], ALU.add, ALU.add)
                                else:
                                    nc.vector.tensor_tensor(acc[m][:], pp[:],
                                                            acc[m][:],
                                                            ALU.add)
                                    nc.vector.tensor_copy(x2b[m][:],
                                                          acc[m][:])
                                    sq2 = cstg.tile([P, TPC], bf16,
                                                    tag="sq2", bufs=3)
                                    nc.vector.tensor_tensor(sq2[:],
                                                            x2b[m][:],
                                                            x2b[m][:],
                                                            ALU.mult)
                                    nc.tensor.matmul(st2x[:], ones_colb[:],
                                                     x2b[m][:],
                                                     start=(m == 0),
                                                     stop=(m == KT - 1))
                                    nc.tensor.matmul(st2q[:], ones_colb[:],
                                                     sq2[:], start=(m == 0),
                                                     stop=(m == KT - 1))
                        # ln2 row stats
                        ex22 = drows.tile([1, TPC], f32, tag="ex22")
                        mu22 = drows.tile([1, TPC], f32, tag="mu22")
                        var2 = drows.tile([1, TPC], f32, tag="var2")
                        std2 = drows.tile([1, TPC], f32r, tag="std2")
                        rrow2 = drows.tile([1, TPC], f32r, tag="rrow2")
                        nc.vector.tensor_scalar_mul(negmu2[:], st2x[:],
                                                    -1.0 / C)
                        nc.vector.tensor_copy(negmu2h[:], negmu2[:])
                        nc.vector.tensor_scalar_mul(ex22[:], st2q[:],
                                                    1.0 / C)
                        nc.vector.tensor_tensor(mu22[:], negmu2[:],
                                                negmu2[:], ALU.mult)
                        nc.vector.tensor_tensor(var2[:], ex22[:], mu22[:],
                                                ALU.subtract)
                        nc.scalar.activation(std2[:], var2[:], AF.Sqrt,
                                             bias=eps_col[0:1, :])
                        nc.vector.reciprocal(rrow2[:], std2[:])
                        rb2p = pbc2.tile([P, TPC], f32, tag="bc2")
                        nc.tensor.matmul(rb2p[:], rr(ones_row[:]),
                                         rr(rrow2[:]), start=True, stop=True)
                        nc.scalar.copy(r2b_s[:], rb2p[:])

                    # ===================== PHASE D: MLP =====================
                    with (
                        tc.tile_pool(name="wfpool", bufs=3) as wfp,
                        tc.tile_pool(name="wgpool", bufs=3) as wgp,
                        tc.tile_pool(name="apool", bufs=1) as apool,
                        tc.tile_pool(name="dstage", bufs=3) as dstg,
                        tc.tile_pool(name="ps_f", bufs=2, space="PSUM") as pf,
                        tc.tile_pool(name="ps_g", bufs=3, space="PSUM") as pg,
                    ):
                        csfc_s = drows.tile([1, FF], bf16, tag="csfc")
                        nc.sync.dma_start(out=csfc_s[:], in_=csfc[:, :])
                        for ch in range(NCH):
                            aT = [apool.tile([P, TPC], bf16, tag=f"aT{m}",
                                             name=f"aT{ch}_{m}", bufs=2)
                                  for m in range(8)]
                            for m in range(8):
                                wfm = wfp.tile([P, KT * P], bf16, tag="wfm",
                                               name=f"wfm{ch}_{m}")
                                nc.sync.dma_start(out=wfm[:],
                                                  in_=wfc[ch * 8 + m, :, :])
                                pft = pf.tile([P, TPC], f32, tag="pf")
                                for k in range(KT):
                                    nc.tensor.matmul(
                                        pft[:], wfm[:, k * P:(k + 1) * P],
                                        x2b[k][:], start=(k == 0),
                                        stop=False)
                                f0 = (ch * 8 + m) * P
                                nc.tensor.matmul(
                                    pft[:], csfc_s[0:1, f0:f0 + P],
                                    negmu2h[:], start=False, stop=True)
                                tmp = dstg.tile([P, TPC], f32, tag="tmp",
                                                bufs=3)
                                nc.vector.tensor_tensor(tmp[:], pft[:],
                                                        r2b_s[:], ALU.mult)
                                nc.scalar.activation(
                                    aT[m][:], tmp[:], AF.Gelu,
                                    bias=bfcc_s[:, ch * 8 + m:ch * 8 + m + 1])
                            for m in range(KT):
                                wgm = wgp.tile([P, 8 * P], bf16, tag="wgm",
                                               name=f"wgm{ch}_{m}")
                                nc.sync.dma_start(out=wgm[:],
                                                  in_=wfc2[ch, m, :, :])
                                pgt = pg.tile([P, TPC], f32, tag="pg")
                                for kk in range(8):
                                    nc.tensor.matmul(
                                        pgt[:], wgm[:, kk * P:(kk + 1) * P],
                                        aT[kk][:], start=(kk == 0),
                                        stop=(kk == 7))
                                if ch == 0:
                                    nc.vector.scalar_tensor_tensor(
                                        acc[m][:], pgt[:],
                                        bf2c_s[:, m:m + 1], acc[m][:],
                                        ALU.add, ALU.add)
                                else:
                                    nc.vector.tensor_tensor(
                                        acc[m][:], pgt[:], acc[m][:],
                                        ALU.add)
                                if ch == NCH - 1:
                                    nc.sync.dma_start(
                                        out=out[m * P:(m + 1) * P, :],
                                        in_=acc[m][:])

    nc.compile()
    return nc


def _get_program(n_cores, gb1):
    key = (n_cores, gb1)
    if key not in _BUILD_CACHE:
        _BUILD_CACHE[key] = _build_program(n_cores, gb1)
    return _BUILD_CACHE[key]


def _colmajor(v, kt):
    """(kt*128,) vector -> (128, kt) column-tile layout."""
    return np.ascontiguousarray(v.reshape(kt, P).T)


def make_in_maps(x, ln1_w, ln1_b, w_qkv, b_qkv, w_proj, b_proj,
                 ln2_w, ln2_b, w_fc, b_fc, w_fc2, b_fc2, n_cores=N_CORES):
    """Host-side sharding: slicing / transpose / fold / reshape only."""
    f = np.float32
    bf = np.dtype("bfloat16") if hasattr(np, "bfloat16") else None
    import ml_dtypes
    bf = ml_dtypes.bfloat16
    x2d = np.ascontiguousarray(np.asarray(x, f).reshape(TOK, C))
    xT = np.ascontiguousarray(x2d.T)

    # fold ln weights into the projection weights (host-side)
    w_qkv_e = np.asarray(ln1_w, f)[:, None] * np.asarray(w_qkv, f)
    w_fc_e = np.asarray(ln2_w, f)[:, None] * np.asarray(w_fc, f)
    bq_e = np.asarray(ln1_b, f) @ w_qkv_e + np.asarray(b_qkv, f)
    bfc_e = np.asarray(ln2_b, f) @ w_fc_e + np.asarray(b_fc, f)
    gb1 = bool(np.any(bq_e != 0.0))

    # causal mask pair-tiles: mask[d][kk, i*512+qq] = 1 if qq - kk - 128*(2d+i) >= 0
    _kk = np.arange(P)[:, None]
    _qq = np.arange(512)[None, :]
    _m4 = [(_qq - _kk - P * d >= 0).astype(f) for d in range(4)]
    _masks = np.concatenate(
        [np.concatenate([_m4[2 * d], _m4[2 * d + 1]], axis=1)
         for d in range(2)], axis=0).astype(bf)

    wfc_t = np.ascontiguousarray(
        w_fc_e.reshape(KT, P, FF // P, P)
        .transpose(2, 1, 0, 3).reshape(FF // P, P, KT * P)).astype(bf)
    wfc2_t = np.ascontiguousarray(
        np.asarray(w_fc2, f).reshape(8, 8, P, KT, P)
        .transpose(0, 3, 2, 1, 4).reshape(8, KT, P, 8 * P)).astype(bf)
    # x, transposed, chunk-major [NCH, P, KT*512]
    xT_t = np.ascontiguousarray(
        xT.reshape(KT, P, NCH, 512).transpose(2, 1, 0, 3)
        .reshape(NCH, P, KT * 512)).astype(bf)
    # w_proj rows grouped by (hl, j): block hl*8+j = head (2j+hl)
    wp = np.asarray(w_proj, f).reshape(H, P, C)
    wpj_t = np.ascontiguousarray(
        np.stack([wp[2 * j + hl] for hl in range(2) for j in range(NCH)],
                 axis=0)).astype(bf)
    csfc_r = w_fc_e.sum(axis=0)[None, :].astype(bf)

    shared = {
        "xTt": xT_t,
        "ones_f": np.ones((P, P), f),
        "ones_b": np.ones((P, P), bf),
        "masks_in": _masks,
        "wpj": wpj_t,
        "wfc": wfc_t,
        "csfc": csfc_r,
        "wfc2": wfc2_t,
        "bpjc": _colmajor(np.asarray(b_proj, f), KT),
        "bfcc": _colmajor(bfc_e, FF // P),
        "bf2c": _colmajor(np.asarray(b_fc2, f), KT),
    }
    in_maps = []
    for c in range(n_cores):
        m = dict(shared)
        m["xTm"] = np.ascontiguousarray(xT[:, c * TPC:(c + 1) * TPC])

        def _kpf(w):  # (C, FW) -> (P p, KT*FW kf)
            return np.ascontiguousarray(
                w.reshape(KT, P, FW).transpose(1, 0, 2).reshape(P, KT * FW))
        wqc = w_qkv_e[:, c * FW:(c + 1) * FW]
        wkc = w_qkv_e[:, C + c * FW:C + (c + 1) * FW]
        wvc = w_qkv_e[:, 2 * C + c * FW:2 * C + (c + 1) * FW]
        m["wq"] = _kpf(wqc).astype(bf)
        m["wk"] = _kpf(wkc).astype(bf)
        m["wv"] = _kpf(wvc).astype(bf)
        m["csqkv"] = np.concatenate(
            [wqc.sum(axis=0), wkc.sum(axis=0), wvc.sum(axis=0)])[None, :]\
            .astype(bf)
        if gb1:
            m["bqr"] = bq_e[None, c * FW:(c + 1) * FW].astype(bf)
            m["bkr"] = bq_e[None, C + c * FW:C + (c + 1) * FW].astype(bf)
            m["bvr"] = bq_e[None, 2 * C + c * FW:2 * C + (c + 1) * FW].astype(bf)
        in_maps.append(m)
    return in_maps, gb1


def kernel(**inputs):
    from concourse.bass_utils import run_bass_kernel_spmd

    in_maps, gb1 = make_in_maps(**inputs)
    nc = _get_program(N_CORES, gb1)

    trace = os.environ.get("KERNEL_TRACE", "0") == "1"
    kw = {}
    if trace:
        kw = dict(trace=True)
    try:
        res = run_bass_kernel_spmd(nc, in_maps, list(range(N_CORES)), **kw)
    except Exception as e:
        if not trace:
            raise
        _LAST_RESULTS["trace_error"] = repr(e)
        res = run_bass_kernel_spmd(nc, in_maps, list(range(N_CORES)))
    _LAST_RESULTS["exec_time_ns"] = res.exec_time_ns
    _LAST_RESULTS["mean_exec_time_ns"] = res.mean_exec_time_ns
    _LAST_RESULTS["results"] = res
    outT = np.concatenate([res.results[i]["out"] for i in range(N_CORES)],
                          axis=1)
    return np.ascontiguousarray(outT.T).reshape(B, T, C).astype(np.float32)


# revision 17
# speedup vs baseline: 1.3004x; 1.0201x over previous
"""Trainium2 Bass kernel for nn_Block_28887950033544 (dense transformer block).

Shapes: x (B=2, T=2048, C=2048), H=16 heads, HS=128, MLP hidden 4C=8192.

v2 sharding over 8 NeuronCores:
  - attention: head-parallel (2 heads/core); qkv computed on the full
    4096-token stream per core for the core's heads; q/k/v stay SBUF-resident
    in bf16 (no DRAM roundtrip).
  - after attention, two 1MB AllToAll collectives (one per local-head slot)
    redistribute y from head-sharded to token-sharded layout.
  - proj + MLP: token-parallel (512 tokens/core).

All heavy matmuls run in bf16 (inputs are bf16; PSUM accumulation is fp32).
LayerNorm centering is folded into the matmuls as rank-1 corrections
(colsum(W) x mu), so the projections never wait on the stats.
ln1_w/ln2_w are folded into the weights host-side; ln biases fold into
host-precomputed bias columns/rows.

Everything on device runs in transposed activation layout (C x tokens).
"""

import os
import sys

for _p in ("/opt/trn_rl_repo",):
    if _p not in sys.path and os.path.isdir(_p):
        sys.path.insert(0, _p)

import numpy as np

# --- problem constants (hardcoded per contract) ---
B, T, C, H = 2, 2048, 2048, 16
HS = C // H          # 128
TOK = B * T          # 4096
P = 128              # partitions
KT = C // P          # 16 k-tiles over C
NCH = TOK // 512     # 8 token chunks of 512
FF = 4 * C           # 8192
EPS = 1e-5
ISQ = float(1.0 / np.sqrt(HS))
N_CORES = 8
TPC = TOK // N_CORES   # 512 tokens per core (proj/MLP slice)
HPC = H // N_CORES     # 2 heads per core
FW = HPC * HS          # 256

_BUILD_CACHE = {}
_LAST_RESULTS = {"exec_time_ns": None, "mean_exec_time_ns": None}


def _build_program(n_cores, gb1):
    """Build the (SPMD, per-core identical) Bass/Tile program.

    gb1: general-path flag for a nontrivial fused qkv bias
    (ln1_b @ W + b_qkv != 0).  The harness inputs have zero biases, so the
    specialized path is the one that actually runs.
    """
    from concourse import bacc
    import concourse.mybir as mybir
    import concourse.tile as tile

    dt = mybir.dt
    f32 = dt.float32
    f32r = dt.float32r
    bf16 = dt.bfloat16
    AF = mybir.ActivationFunctionType
    ALU = mybir.AluOpType

    nc = bacc.Bacc("TRN2", target_bir_lowering=False, debug=False,
                   num_devices=n_cores)

    # ---- DRAM I/O ----
    # full token stream, transposed, chunk-major: [NCH, P, KT*512] bf16
    xTt = nc.dram_tensor("xTt", [NCH, P, KT * 512], bf16,
                         kind="ExternalInput").ap()
    xTm = nc.dram_tensor("xTm", [C, TPC], f32, kind="ExternalInput").ap()
    wq = nc.dram_tensor("wq", [P, KT * FW], bf16, kind="ExternalInput").ap()
    wk = nc.dram_tensor("wk", [P, KT * FW], bf16, kind="ExternalInput").ap()
    wv = nc.dram_tensor("wv", [P, KT * FW], bf16, kind="ExternalInput").ap()
    csqkv = nc.dram_tensor("csqkv", [1, 3 * FW], bf16,
                           kind="ExternalInput").ap()
    wpj = nc.dram_tensor("wpj", [2 * NCH, P, C], bf16,
                         kind="ExternalInput").ap()   # [hl*8+j] head (2j+hl)
    wfc = nc.dram_tensor("wfc", [FF // P, P, KT * P], bf16,
                         kind="ExternalInput").ap()
    csfc = nc.dram_tensor("csfc", [1, FF], bf16, kind="ExternalInput").ap()
    wfc2 = nc.dram_tensor("wfc2", [NCH, KT, P, NCH * P], bf16,
                          kind="ExternalInput").ap()
    bpjc = nc.dram_tensor("bpjc", [P, KT], f32, kind="ExternalInput").ap()
    bfcc = nc.dram_tensor("bfcc", [P, FF // P], f32, kind="ExternalInput").ap()
    bf2c = nc.dram_tensor("bf2c", [P, KT], f32, kind="ExternalInput").ap()
    ones_f = nc.dram_tensor("ones_f", [P, P], f32, kind="ExternalInput").ap()
    ones_b = nc.dram_tensor("ones_b", [P, P], bf16, kind="ExternalInput").ap()
    masks_in = nc.dram_tensor("masks_in", [2 * P, 1024], bf16,
                              kind="ExternalInput").ap()
    if gb1:
        bqr = nc.dram_tensor("bqr", [1, FW], bf16, kind="ExternalInput").ap()
        bkr = nc.dram_tensor("bkr", [1, FW], bf16, kind="ExternalInput").ap()
        bvr = nc.dram_tensor("bvr", [1, FW], bf16, kind="ExternalInput").ap()
    out = nc.dram_tensor("out", [C, TPC], f32, kind="ExternalOutput").ap()

    def rr(ap):
        return ap.bitcast(f32r)

    with tile.TileContext(nc) as tc, \
         nc.allow_low_precision(reason="bf16 matmul inputs; all matmul "
                                "accumulation stays fp32 in PSUM"):
        with tc.tile_pool(name="dram", bufs=1, space="DRAM") as dram:
            a2a_in = [dram.tile([NCH * P, 512], bf16, name=f"a2a_in{hl}")
                      for hl in range(2)]
            a2a_out = [dram.tile([NCH * P, 512], bf16, name=f"a2a_out{hl}")
                       for hl in range(2)]

            with tc.tile_pool(name="const", bufs=1) as const:
                ones_colb = const.tile([P, 1], bf16)    # stats lhsT
                nc.sync.dma_start(out=ones_colb[:], in_=ones_b[:, 0:1])
                ones_row = const.tile([1, P], f32r)     # f32r bcast lhsT
                nc.sync.dma_start(out=ones_row[:],
                                  in_=ones_f[0:1, :].bitcast(f32r))
                eps_col = const.tile([P, 1], f32)
                nc.vector.memset(eps_col[:], EPS)
                # persistent bf16 activations (SBUF-resident across phases)
                with tc.tile_pool(name="qkv_sb", bufs=1) as qkvp:
                    qT_sb = [qkvp.tile([P, TOK], bf16, name=f"qT{m}")
                             for m in range(HPC)]
                    kT_sb = [qkvp.tile([P, TOK], bf16, name=f"kT{m}")
                            for m in range(HPC)]
                    v_sb = [qkvp.tile([P, FW], bf16, name=f"v{i}")
                            for i in range(TOK // P)]

                    # ================= PHASE A: ln1 + qkv =================
                    with (
                        tc.tile_pool(name="wqkv", bufs=1) as wpool,
                        tc.tile_pool(name="xchunk", bufs=2) as xpool,
                        tc.tile_pool(name="arows", bufs=2) as rows,
                        tc.tile_pool(name="astage", bufs=1) as stg,
                        tc.tile_pool(name="ps_st", bufs=1, space="PSUM") as pst,
                        tc.tile_pool(name="ps_bc", bufs=1, space="PSUM") as pbc,
                        tc.tile_pool(name="ps_qk", bufs=3, space="PSUM") as pqk,
                        tc.tile_pool(name="ps_v", bufs=2, space="PSUM") as pv,
                    ):
                        # chunk-0 x first so stats matmuls start ASAP
                        xb0 = xpool.tile([P, KT * 512], bf16, tag="xb",
                                         name="xb0")
                        nc.sync.dma_start(out=xb0[:], in_=xTt[0, :, :])
                        wq_s = wpool.tile([P, KT * FW], bf16, tag="wq")
                        wk_s = wpool.tile([P, KT * FW], bf16, tag="wk")
                        wv_s = wpool.tile([P, KT * FW], bf16, tag="wv")
                        nc.sync.dma_start(out=wq_s[:], in_=wq[:, :])
                        nc.sync.dma_start(out=wk_s[:], in_=wk[:, :])
                        nc.sync.dma_start(out=wv_s[:], in_=wv[:, :])
                        # remaining constants (not needed by the first mms)
                        ones_sqb = const.tile([P, P], bf16)     # denom lhsT
                        nc.sync.dma_start(out=ones_sqb[:], in_=ones_b[:, :])
                        masks = []
                        for d in range(2):
                            m = const.tile([P, 1024], bf16, name=f"mask{d}")
                            nc.sync.dma_start(
                                out=m[:], in_=masks_in[d * P:(d + 1) * P, :])
                            masks.append(m)
                        bpjc_s = const.tile([P, KT], f32)
                        nc.sync.dma_start(out=bpjc_s[:], in_=bpjc[:, :])
                        bfcc_s = const.tile([P, FF // P], f32)
                        nc.sync.dma_start(out=bfcc_s[:], in_=bfcc[:, :])
                        bf2c_s = const.tile([P, KT], f32)
                        nc.sync.dma_start(out=bf2c_s[:], in_=bf2c[:, :])
                        csq_s = const.tile([1, 3 * FW], bf16)
                        nc.sync.dma_start(out=csq_s[:], in_=csqkv[:, :])
                        if gb1:
                            b_rows = {}
                            for nm, src in (("q", bqr), ("k", bkr),
                                            ("v", bvr)):
                                t = const.tile([1, FW], bf16,
                                               name=f"brow_{nm}")
                                nc.sync.dma_start(out=t[:], in_=src[:, :])
                                b_rows[nm] = t

                        for c in range(NCH):
                            tok0 = c * 512
                            if c == 0:
                                xb = xb0
                            else:
                                xb = xpool.tile([P, KT * 512], bf16,
                                                tag="xb", name=f"xb{c}")
                                nc.sync.dma_start(out=xb[:], in_=xTt[c, :, :])
                            xk = [xb[:, k * 512:(k + 1) * 512]
                                  for k in range(KT)]
                            # --- stats ---
                            stx = pst.tile([1, 512], f32, tag="stx")
                            stq = pst.tile([1, 512], f32, tag="stq")
                            for k in range(KT):
                                sq = stg.tile([P, 512], bf16, tag="sq",
                                              bufs=3)
                                nc.vector.tensor_tensor(sq[:], xk[k], xk[k],
                                                        ALU.mult)
                                nc.tensor.matmul(stx[:], ones_colb[:], xk[k],
                                                 start=(k == 0),
                                                 stop=(k == KT - 1))
                                nc.tensor.matmul(stq[:], ones_colb[:], sq[:],
                                                 start=(k == 0),
                                                 stop=(k == KT - 1))
                            negmu = rows.tile([1, 512], f32r, tag="negmu")
                            negmuh = rows.tile([1, 512], bf16, tag="negmuh")
                            ex2 = rows.tile([1, 512], f32, tag="ex2")
                            mu2 = rows.tile([1, 512], f32, tag="mu2")
                            var = rows.tile([1, 512], f32, tag="var")
                            std = rows.tile([1, 512], f32r, tag="std")
                            rrow = rows.tile([1, 512], f32r, tag="rrow")
                            if gb1:
                                stdh = rows.tile([1, 512], bf16, tag="stdh")
                            nc.vector.tensor_scalar_mul(negmu[:], stx[:],
                                                        -1.0 / C)
                            nc.vector.tensor_copy(negmuh[:], negmu[:])
                            nc.vector.tensor_scalar_mul(ex2[:], stq[:],
                                                        1.0 / C)
                            nc.vector.tensor_tensor(mu2[:], negmu[:],
                                                    negmu[:], ALU.mult)
                            nc.vector.tensor_tensor(var[:], ex2[:], mu2[:],
                                                    ALU.subtract)
                            nc.scalar.activation(std[:], var[:], AF.Sqrt,
                                                 bias=eps_col[0:1, :])
                            nc.vector.reciprocal(rrow[:], std[:])
                            if gb1:
                                nc.vector.tensor_copy(stdh[:], std[:])
                            # r as column form (4 outer products) + bcast
                            rcolp = pbc.tile([P, 4], f32, tag="rbp",
                                             name=f"rcolp{c}")
                            for m in range(4):
                                nc.tensor.matmul(
                                    rcolp[:, m:m + 1],
                                    rrow[0:1, m * P:(m + 1) * P].bitcast(f32),
                                    ones_row[0:1, 0:1].bitcast(f32),
                                    start=True, stop=True)
                            rcol = rows.tile([P, 4], f32, tag="rcol")
                            nc.scalar.copy(rcol[:], rcolp[:])
                            rbp = pbc.tile([P, 512], f32, tag="rbp")
                            nc.tensor.matmul(rbp[:], rr(ones_row[:]),
                                             rr(rrow[:]), start=True,
                                             stop=True)
                            rb_s = stg.tile([P, 512], f32, tag="rb", bufs=2)
                            nc.scalar.copy(rb_s[:], rbp[:])

                            # Q^T, K^T (feat x tok) on raw x + rank-1 fix
                            for qk_i, (ws, dst) in enumerate(
                                    ((wq_s, qT_sb), (wk_s, kT_sb))):
                                for m in range(HPC):
                                    pq = pqk.tile([P, 512], f32, tag="pqk")
                                    for k in range(KT):
                                        nc.tensor.matmul(
                                            pq[:],
                                            ws[:, k * FW + m * P:
                                               k * FW + (m + 1) * P],
                                            xk[k],
                                            start=(k == 0), stop=False)
                                    cs0 = qk_i * FW + m * P
                                    nc.tensor.matmul(
                                        pq[:], csq_s[0:1, cs0:cs0 + P],
                                        negmuh[:], start=False,
                                        stop=(not gb1))
                                    if gb1:
                                        br = b_rows["q" if qk_i == 0 else "k"]
                                        nc.tensor.matmul(
                                            pq[:], br[0:1, m * P:(m + 1) * P],
                                            stdh[:], start=False, stop=True)
                                    nc.vector.tensor_tensor(
                                        dst[m][:, tok0:tok0 + 512],
                                        pq[:], rb_s[:], ALU.mult)
                            # V (tok x feat) on raw x + rank-1 fix
                            for mt in range(4):
                                pvt = pv.tile([P, FW], f32, tag="pv")
                                for k in range(KT):
                                    nc.tensor.matmul(
                                        pvt[:],
                                        xk[k][:, mt * P:(mt + 1) * P],
                                        wv_s[:, k * FW:(k + 1) * FW],
                                        start=(k == 0), stop=False)
                                nc.tensor.matmul(
                                    pvt[:],
                                    negmuh[0:1, mt * P:(mt + 1) * P],
                                    csq_s[0:1, 2 * FW:3 * FW],
                                    start=False, stop=(not gb1))
                                if gb1:
                                    nc.tensor.matmul(
                                        pvt[:],
                                        stdh[0:1, mt * P:(mt + 1) * P],
                                        b_rows["v"][:],
                                        start=False, stop=True)
                                nc.scalar.activation(
                                    v_sb[c * 4 + mt][:], pvt[:], AF.Copy,
                                    scale=rcol[:, mt:mt + 1])

                    # ================= PHASE B: attention =================
                    with (
                        tc.tile_pool(name="expp", bufs=3) as ep,
                        tc.tile_pool(name="bstage", bufs=2) as bstg,
                        tc.tile_pool(name="ystage", bufs=2) as ystg,
                        tc.tile_pool(name="ps_sc", bufs=2, space="PSUM") as psc,
                        tc.tile_pool(name="ps_dn", bufs=2, space="PSUM") as pdn,
                        tc.tile_pool(name="ps_y", bufs=2, space="PSUM") as psy,
                    ):
                        for u, (hl, bb) in enumerate(
                                ((0, 0), (0, 1), (1, 0), (1, 1))):
                            qhb = qT_sb[hl][:, bb * T:(bb + 1) * T]
                            khb = kT_sb[hl][:, bb * T:(bb + 1) * T]
                            yT = ystg.tile([P, T], bf16, tag="yT",
                                           name=f"yT{u}")
                            for qc in range(T // 512):
                                nk = 4 * (qc + 1)
                                ebigs = []
                                for g in range(nk // 2):
                                    ps = psc.tile([P, 1024], f32, tag="sc")
                                    for i in range(2):
                                        kt = 2 * g + i
                                        nc.tensor.matmul(
                                            ps[:, i * 512:(i + 1) * 512],
                                            khb[:, kt * P:(kt + 1) * P],
                                            qhb[:, qc * 512:(qc + 1) * 512],
                                            start=True, stop=True)
                                    e = ep.tile([P, 1024], bf16, tag="e",
                                                name=f"e{g}", bufs=4)
                                    if 2 * g >= 4 * qc:
                                        etmp = bstg.tile([P, 1024], bf16,
                                                         tag="ed", bufs=2)
                                        nc.scalar.activation(etmp[:], ps[:],
                                                             AF.Exp,
                                                             scale=ISQ)
                                        nc.vector.tensor_tensor(
                                            e[:], etmp[:],
                                            masks[(2 * g - 4 * qc) // 2][:],
                                            ALU.mult)
                                    else:
                                        nc.scalar.activation(e[:], ps[:],
                                                             AF.Exp,
                                                             scale=ISQ)
                                    ebigs.append(e)
                                pd = pdn.tile([P, 512], f32, tag="pd")
                                py = psy.tile([P, 512], f32, tag="py")
                                for kt in range(nk):
                                    sl = ebigs[kt // 2][:, (kt % 2) * 512:
                                                        (kt % 2) * 512 + 512]
                                    nc.tensor.matmul(pd[:], ones_sqb[:], sl,
                                                     start=(kt == 0),
                                                     stop=(kt == nk - 1))
                                    vt = v_sb[bb * 16 + kt]
                                    nc.tensor.matmul(
                                        py[:], vt[:, hl * P:(hl + 1) * P], sl,
                                        start=(kt == 0), stop=(kt == nk - 1))
                                rc = bstg.tile([P, 512], f32, tag="rc",
                                               bufs=2)
                                nc.vector.reciprocal(rc[:], pd[:])
                                nc.vector.tensor_tensor(
                                    yT[:, qc * 512:(qc + 1) * 512],
                                    py[:], rc[:], ALU.mult)
                            for j in range(4):
                                nc.sync.dma_start(
                                    out=a2a_in[hl][(bb * 4 + j) * P:
                                                   (bb * 4 + j + 1) * P, :],
                                    in_=yT[:, j * 512:(j + 1) * 512])
                            if bb == 1:
                                if n_cores > 1:
                                    nc.gpsimd.collective_compute(
                                        "AllToAll", ALU.bypass,
                                        replica_groups=[list(range(n_cores))],
                                        ins=[a2a_in[hl][:, :].opt()],
                                        outs=[a2a_out[hl][:, :].opt()],
                                    )
                                else:
                                    nc.sync.dma_start(out=a2a_out[hl][:, :],
                                                      in_=a2a_in[hl][:, :])

                # =============== PHASE C: proj (own tokens) ===============
                with (
                    tc.tile_pool(name="x2pool", bufs=1) as x2p,
                    tc.tile_pool(name="drows", bufs=1) as drows,
                ):
                    acc = [x2p.tile([P, TPC], f32, name=f"acc{m}")
                           for m in range(KT)]
                    x2b = [x2p.tile([P, TPC], bf16, name=f"x2b{m}")
                           for m in range(KT)]
                    negmu2 = drows.tile([1, TPC], f32r, tag="negmu2")
                    negmu2h = drows.tile([1, TPC], bf16, tag="negmu2h")
                    r2b_s = drows.tile([P, TPC], f32, tag="r2b")
                    with (
                        tc.tile_pool(name="wpj_p", bufs=1) as wpp,
                        tc.tile_pool(name="ygp", bufs=1) as ygp,
                        tc.tile_pool(name="cstage", bufs=2) as cstg,
                        tc.tile_pool(name="ps_pj", bufs=3, space="PSUM") as ppj,
                        tc.tile_pool(name="ps_st2", bufs=1,
                                     space="PSUM") as pst2,
                        tc.tile_pool(name="ps_bc2", bufs=1,
                                     space="PSUM") as pbc2,
                    ):
                        st2x = pst2.tile([1, TPC], f32, tag="st2x")
                        st2q = pst2.tile([1, TPC], f32, tag="st2q")
                        wpj_s = {}
                        yg = {}

                        def _load_pj(hl):
                            for j in range(NCH):
                                w = wpp.tile([P, C], bf16, tag=f"wpj{hl}_{j}",
                                             name=f"wpj{hl}_{j}")
                                nc.sync.dma_start(out=w[:],
                                                  in_=wpj[hl * NCH + j, :, :])
                                wpj_s[(hl, j)] = w
                            for j in range(NCH):
                                y = ygp.tile([P, 512], bf16,
                                             tag=f"yg{hl}_{j}",
                                             name=f"yg{hl}_{j}")
                                nc.sync.dma_start(
                                    out=y[:],
                                    in_=a2a_out[hl][j * P:(j + 1) * P, :])
                                yg[(hl, j)] = y

                        _load_pj(0)
                        for hl in range(2):
                            if hl == 1:
                                # deferred: keeps the pass-0 xmy loads ahead
                                # of the A2A_1-gated reads in the DMA FIFO
                                _load_pj(1)
                            for m in range(KT):
                                pp = ppj.tile([P, TPC], f32, tag="pp")
                                for j in range(NCH):
                                    nc.tensor.matmul(
                                        pp[:],
                                        wpj_s[(hl, j)][:, m * P:(m + 1) * P],
                                        yg[(hl, j)][:],
                                        start=(j == 0), stop=(j == NCH - 1))
                                if hl == 0:
                                    xmy = cstg.tile([P, TPC], f32, tag="xmy",
                                                    bufs=3)
                                    nc.sync.dma_start(
                                        out=xmy[:],
                                        in_=xTm[m * P:(m + 1) * P, :])
                                    nc.vector.scalar_tensor_tensor(
                                        acc[m][:], pp[:], bpjc_s[:, m:m + 1],
                                        xmy[:], ALU.add, ALU.add)
                                else:
                                    nc.vector.tensor_tensor(acc[m][:], pp[:],
                                                            acc[m][:],
                                                            ALU.add)
                                    nc.vector.tensor_copy(x2b[m][:],
                                                          acc[m][:])
                                    sq2 = cstg.tile([P, TPC], bf16,
                                                    tag="sq2", bufs=3)
                                    nc.vector.tensor_tensor(sq2[:],
                                                            x2b[m][:],
                                                            x2b[m][:],
                                                            ALU.mult)
                                    nc.tensor.matmul(st2x[:], ones_colb[:],
                                                     x2b[m][:],
                                                     start=(m == 0),
                                                     stop=(m == KT - 1))
                                    nc.tensor.matmul(st2q[:], ones_colb[:],
                                                     sq2[:], start=(m == 0),
                                                     stop=(m == KT - 1))
                        # ln2 row stats
                        ex22 = drows.tile([1, TPC], f32, tag="ex22")
                        mu22 = drows.tile([1, TPC], f32, tag="mu22")
                        var2 = drows.tile([1, TPC], f32, tag="var2")
                        std2 = drows.tile([1, TPC], f32r, tag="std2")
                        rrow2 = drows.tile([1, TPC], f32r, tag="rrow2")
                        nc.vector.tensor_scalar_mul(negmu2[:], st2x[:],
                                                    -1.0 / C)
                        nc.vector.tensor_copy(negmu2h[:], negmu2[:])
                        nc.vector.tensor_scalar_mul(ex22[:], st2q[:],
                                                    1.0 / C)
                        nc.vector.tensor_tensor(mu22[:], negmu2[:],
                                                negmu2[:], ALU.mult)
                        nc.vector.tensor_tensor(var2[:], ex22[:], mu22[:],
                                                ALU.subtract)
                        nc.scalar.activation(std2[:], var2[:], AF.Sqrt,
                                             bias=eps_col[0:1, :])
                        nc.vector.reciprocal(rrow2[:], std2[:])
                        rb2p = pbc2.tile([P, TPC], f32, tag="bc2")
                        nc.tensor.matmul(rb2p[:], rr(ones_row[:]),
                                         rr(rrow2[:]), start=True, stop=True)
                        nc.scalar.copy(r2b_s[:], rb2p[:])

                    # ===================== PHASE D: MLP =====================
                    with (
                        tc.tile_pool(name="wfpool", bufs=3) as wfp,
                        tc.tile_pool(name="wgpool", bufs=3) as wgp,
                        tc.tile_pool(name="apool", bufs=1) as apool,
                        tc.tile_pool(name="dstage", bufs=3) as dstg,
                        tc.tile_pool(name="ps_f", bufs=2, space="PSUM") as pf,
                        tc.tile_pool(name="ps_g", bufs=3, space="PSUM") as pg,
                    ):
                        csfc_s = drows.tile([1, FF], bf16, tag="csfc")
                        nc.sync.dma_start(out=csfc_s[:], in_=csfc[:, :])
                        for ch in range(NCH):
                            aT = [apool.tile([P, TPC], bf16, tag=f"aT{m}",
                                             name=f"aT{ch}_{m}", bufs=2)
                                  for m in range(8)]
                            for m in range(8):
                                wfm = wfp.tile([P, KT * P], bf16, tag="wfm",
                                               name=f"wfm{ch}_{m}")
                                nc.sync.dma_start(out=wfm[:],
                                                  in_=wfc[ch * 8 + m, :, :])
                                pft = pf.tile([P, TPC], f32, tag="pf")
                                for k in range(KT):
                                    nc.tensor.matmul(
                                        pft[:], wfm[:, k * P:(k + 1) * P],
                                        x2b[k][:], start=(k == 0),
                                        stop=False)
                                f0 = (ch * 8 + m) * P
                                nc.tensor.matmul(
                                    pft[:], csfc_s[0:1, f0:f0 + P],
                                    negmu2h[:], start=False, stop=True)
                                tmp = dstg.tile([P, TPC], f32, tag="tmp",
                                                bufs=3)
                                nc.vector.tensor_tensor(tmp[:], pft[:],
                                                        r2b_s[:], ALU.mult)
                                nc.scalar.activation(
                                    aT[m][:], tmp[:], AF.Gelu,
                                    bias=bfcc_s[:, ch * 8 + m:ch * 8 + m + 1])
                            for m in range(KT):
                                wgm = wgp.tile([P, 8 * P], bf16, tag="wgm",
                                               name=f"wgm{ch}_{m}")
                                nc.sync.dma_start(out=wgm[:],
                                                  in_=wfc2[ch, m, :, :])
                                pgt = pg.tile([P, TPC], f32, tag="pg")
                                for kk in range(8):
                                    nc.tensor.matmul(
                                        pgt[:], wgm[:, kk * P:(kk + 1) * P],
                                        aT[kk][:], start=(kk == 0),
                                        stop=(kk == 7))
                                if ch == 0:
                                    nc.vector.scalar_tensor_tensor(
                                        acc[m][:], pgt[:],
                                        bf2c_s[:, m:m + 1], acc[m][:],
                                        ALU.add, ALU.add)
                                else:
                                    nc.vector.tensor_tensor(
                                        acc[m][:], pgt[:], acc[m][:],
                                        ALU.add)
                                if ch == NCH - 1:
                                    nc.sync.dma_start(
                                        out=out[m * P:(m + 1) * P, :],
                                        in_=acc[m][:])

    nc.compile()
    return nc


def _get_program(n_cores, gb1):
    key = (n_cores, gb1)
    if key not in _BUILD_CACHE:
        _BUILD_CACHE[key] = _build_program(n_cores, gb1)
    return _BUILD_CACHE[key]


def _colmajor(v, kt):
    """(kt*128,) vector -> (128, kt) column-tile layout."""
    return np.ascontiguousarray(v.reshape(kt, P).T)


def make_in_maps(x, ln1_w, ln1_b, w_qkv, b_qkv, w_proj, b_proj,
                 ln2_w, ln2_b, w_fc, b_fc, w_fc2, b_fc2, n_cores=N_CORES):
    """Host-side sharding: slicing / transpose / fold / reshape only."""
    f = np.float32
    bf = np.dtype("bfloat16") if hasattr(np, "bfloat16") else None
    import ml_dtypes
    bf = ml_dtypes.bfloat16
    x2d = np.ascontiguousarray(np.asarray(x, f).reshape(TOK, C))
    xT = np.ascontiguousarray(x2d.T)

    # fold ln weights into the projection weights (host-side)
    w_qkv_e = np.asarray(ln1_w, f)[:, None] * np.asarray(w_qkv, f)
    w_fc_e = np.asarray(ln2_w, f)[:, None] * np.asarray(w_fc, f)
    bq_e = np.asarray(ln1_b, f) @ w_qkv_e + np.asarray(b_qkv, f)
    bfc_e = np.asarray(ln2_b, f) @ w_fc_e + np.asarray(b_fc, f)
    gb1 = bool(np.any(bq_e != 0.0))

    # causal mask pair-tiles: mask[d][kk, i*512+qq] = 1 if qq - kk - 128*(2d+i) >= 0
    _kk = np.arange(P)[:, None]
    _qq = np.arange(512)[None, :]
    _m4 = [(_qq - _kk - P * d >= 0).astype(f) for d in range(4)]
    _masks = np.concatenate(
        [np.concatenate([_m4[2 * d], _m4[2 * d + 1]], axis=1)
         for d in range(2)], axis=0).astype(bf)

    wfc_t = np.ascontiguousarray(
        w_fc_e.reshape(KT, P, FF // P, P)
        .transpose(2, 1, 0, 3).reshape(FF // P, P, KT * P)).astype(bf)
    wfc2_t = np.ascontiguousarray(
        np.asarray(w_fc2, f).reshape(8, 8, P, KT, P)
        .transpose(0, 3, 2, 1, 4).reshape(8, KT, P, 8 * P)).astype(bf)
    # x, transposed, chunk-major [NCH, P, KT*512]
    xT_t = np.ascontiguousarray(
        xT.reshape(KT, P, NCH, 512).transpose(2, 1, 0, 3)
        .reshape(NCH, P, KT * 512)).astype(bf)
    # w_proj rows grouped by (hl, j): block hl*8+j = head (2j+hl)
    wp = np.asarray(w_proj, f).reshape(H, P, C)
    wpj_t = np.ascontiguousarray(
        np.stack([wp[2 * j + hl] for hl in range(2) for j in range(NCH)],
                 axis=0)).astype(bf)
    csfc_r = w_fc_e.sum(axis=0)[None, :].astype(bf)

    shared = {
        "xTt": xT_t,
        "ones_f": np.ones((P, P), f),
        "ones_b": np.ones((P, P), bf),
        "masks_in": _masks,
        "wpj": wpj_t,
        "wfc": wfc_t,
        "csfc": csfc_r,
        "wfc2": wfc2_t,
        "bpjc": _colmajor(np.asarray(b_proj, f), KT),
        "bfcc": _colmajor(bfc_e, FF // P),
        "bf2c": _colmajor(np.asarray(b_fc2, f), KT),
    }
    in_maps = []
    for c in range(n_cores):
        m = dict(shared)
        m["xTm"] = np.ascontiguousarray(xT[:, c * TPC:(c + 1) * TPC])

        def _kpf(w):  # (C, FW) -> (P p, KT*FW kf)
            return np.ascontiguousarray(
                w.reshape(KT, P, FW).transpose(1, 0, 2).reshape(P, KT * FW))
        wqc = w_qkv_e[:, c * FW:(c + 1) * FW]
        wkc = w_qkv_e[:, C + c * FW:C + (c + 1) * FW]
        wvc = w_qkv_e[:, 2 * C + c * FW:2 * C + (c + 1) * FW]
        m["wq"] = _kpf(wqc).astype(bf)
        m["wk"] = _kpf(wkc).astype(bf)
        m["wv"] = _kpf(wvc).astype(bf)
        m["csqkv"] = np.concatenate(
            [wqc.sum(axis=0), wkc.sum(axis=0), wvc.sum(axis=0)])[None, :]\
            .astype(bf)
        if gb1:
            m["bqr"] = bq_e[None, c * FW:(c + 1) * FW].astype(bf)
            m["bkr"] = bq_e[None, C + c * FW:C + (c + 1) * FW].astype(bf)
            m["bvr"] = bq_e[None, 2 * C + c * FW:2 * C + (c + 1) * FW].astype(bf)
        in_maps.append(m)
    return in_maps, gb1


def kernel(**inputs):
    from concourse.bass_utils import run_bass_kernel_spmd

    in_maps, gb1 = make_in_maps(**inputs)
    nc = _get_program(N_CORES, gb1)

    trace = os.environ.get("KERNEL_TRACE", "0") == "1"
    kw = {}
    if trace:
        kw = dict(trace=True)
    try:
        res = run_bass_kernel_spmd(nc, in_maps, list(range(N_CORES)), **kw)
    except Exception as e:
        if not trace:
            raise
        _LAST_RESULTS["trace_error"] = repr(e)
        res = run_bass_kernel_spmd(nc, in_maps, list(range(N_CORES)))
    _LAST_RESULTS["exec_time_ns"] = res.exec_time_ns
    _LAST_RESULTS["mean_exec_time_ns"] = res.mean_exec_time_ns
    _LAST_RESULTS["results"] = res
    outT = np.concatenate([res.results[i]["out"] for i in range(N_CORES)],
                          axis=1)
    return np.ascontiguousarray(outT.T).reshape(B, T, C).astype(np.float32)


# revision 31
# speedup vs baseline: 1.3447x; 1.0340x over previous
"""Trainium2 Bass kernel for nn_Block_28887950033544 (dense transformer block).

Shapes: x (B=2, T=2048, C=2048), H=16 heads, HS=128, MLP hidden 4C=8192.

v2 sharding over 8 NeuronCores:
  - attention: head-parallel (2 heads/core); qkv computed on the full
    4096-token stream per core for the core's heads; q/k/v stay SBUF-resident
    in bf16 (no DRAM roundtrip).
  - after attention, two 1MB AllToAll collectives (one per local-head slot)
    redistribute y from head-sharded to token-sharded layout.
  - proj + MLP: token-parallel (512 tokens/core).

All heavy matmuls run in bf16 (inputs are bf16; PSUM accumulation is fp32).
LayerNorm centering is folded into the matmuls as rank-1 corrections
(colsum(W) x mu), so the projections never wait on the stats.
ln1_w/ln2_w are folded into the weights host-side; ln biases fold into
host-precomputed bias columns/rows.

Everything on device runs in transposed activation layout (C x tokens).
"""

import os
import sys

for _p in ("/opt/trn_rl_repo",):
    if _p not in sys.path and os.path.isdir(_p):
        sys.path.insert(0, _p)

import numpy as np

# --- problem constants (hardcoded per contract) ---
B, T, C, H = 2, 2048, 2048, 16
HS = C // H          # 128
TOK = B * T          # 4096
P = 128              # partitions
KT = C // P          # 16 k-tiles over C
NCH = TOK // 512     # 8 token chunks of 512
FF = 4 * C           # 8192
EPS = 1e-5
ISQ = float(1.0 / np.sqrt(HS))
N_CORES = 8
TPC = TOK // N_CORES   # 512 tokens per core (proj/MLP slice)
HPC = H // N_CORES     # 2 heads per core
FW = HPC * HS          # 256

_BUILD_CACHE = {}
_LAST_RESULTS = {"exec_time_ns": None, "mean_exec_time_ns": None}


def _build_program(n_cores, gb1):
    """Build the (SPMD, per-core identical) Bass/Tile program.

    gb1: general-path flag for a nontrivial fused qkv bias
    (ln1_b @ W + b_qkv != 0).  The harness inputs have zero biases, so the
    specialized path is the one that actually runs.
    """
    from concourse import bacc
    import concourse.mybir as mybir
    import concourse.tile as tile

    dt = mybir.dt
    f32 = dt.float32
    f32r = dt.float32r
    bf16 = dt.bfloat16
    AF = mybir.ActivationFunctionType
    ALU = mybir.AluOpType

    nc = bacc.Bacc("TRN2", target_bir_lowering=False, debug=False,
                   num_devices=n_cores)

    # ---- DRAM I/O ----
    # full token stream, transposed, chunk-major: [NCH, P, KT*512] bf16
    xTt = nc.dram_tensor("xTt", [NCH, P, KT * 512], bf16,
                         kind="ExternalInput").ap()
    xTm = nc.dram_tensor("xTm", [C, TPC], f32, kind="ExternalInput").ap()
    wq = nc.dram_tensor("wq", [P, KT * FW], bf16, kind="ExternalInput").ap()
    wk = nc.dram_tensor("wk", [P, KT * FW], bf16, kind="ExternalInput").ap()
    wv = nc.dram_tensor("wv", [P, KT * FW], bf16, kind="ExternalInput").ap()
    csqkv = nc.dram_tensor("csqkv", [1, 3 * FW], bf16,
                           kind="ExternalInput").ap()
    wpj = nc.dram_tensor("wpj", [2 * NCH, P, C], bf16,
                         kind="ExternalInput").ap()   # [hl*8+j] head (2j+hl)
    wfc = nc.dram_tensor("wfc", [FF // P, P, KT * P], bf16,
                         kind="ExternalInput").ap()
    csfc = nc.dram_tensor("csfc", [1, FF], bf16, kind="ExternalInput").ap()
    wfc2 = nc.dram_tensor("wfc2", [NCH, KT, P, NCH * P], bf16,
                          kind="ExternalInput").ap()
    bpjc = nc.dram_tensor("bpjc", [P, KT], f32, kind="ExternalInput").ap()
    bfcc = nc.dram_tensor("bfcc", [P, FF // P], f32, kind="ExternalInput").ap()
    bf2c = nc.dram_tensor("bf2c", [P, KT], f32, kind="ExternalInput").ap()
    ones_f = nc.dram_tensor("ones_f", [P, P], f32, kind="ExternalInput").ap()
    ones_b = nc.dram_tensor("ones_b", [P, P], bf16, kind="ExternalInput").ap()
    masks_in = nc.dram_tensor("masks_in", [2 * P, 1024], bf16,
                              kind="ExternalInput").ap()
    if gb1:
        bqr = nc.dram_tensor("bqr", [1, FW], bf16, kind="ExternalInput").ap()
        bkr = nc.dram_tensor("bkr", [1, FW], bf16, kind="ExternalInput").ap()
        bvr = nc.dram_tensor("bvr", [1, FW], bf16, kind="ExternalInput").ap()
    out = nc.dram_tensor("out", [C, TPC], f32, kind="ExternalOutput").ap()

    def rr(ap):
        return ap.bitcast(f32r)

    with tile.TileContext(nc) as tc, \
         nc.allow_low_precision(reason="bf16 matmul inputs; all matmul "
                                "accumulation stays fp32 in PSUM"):
        with tc.tile_pool(name="dram", bufs=1, space="DRAM") as dram:
            a2a_in = [dram.tile([NCH * P, 512], bf16, name=f"a2a_in{hl}")
                      for hl in range(2)]
            a2a_out = [dram.tile([NCH * P, 512], bf16, name=f"a2a_out{hl}")
                       for hl in range(2)]

            with tc.tile_pool(name="const", bufs=1) as const:
                ones_colb = const.tile([P, 1], bf16)    # stats lhsT
                nc.sync.dma_start(out=ones_colb[:], in_=ones_b[:, 0:1])
                ones_row = const.tile([1, P], f32r)     # f32r bcast lhsT
                nc.sync.dma_start(out=ones_row[:],
                                  in_=ones_f[0:1, :].bitcast(f32r))
                eps_col = const.tile([P, 1], f32)
                nc.vector.memset(eps_col[:], EPS)
                # persistent bf16 activations (SBUF-resident across phases)
                with tc.tile_pool(name="qkv_sb", bufs=1) as qkvp:
                    qT_sb = [qkvp.tile([P, TOK], bf16, name=f"qT{m}")
                             for m in range(HPC)]
                    kT_sb = [qkvp.tile([P, TOK], bf16, name=f"kT{m}")
                            for m in range(HPC)]
                    v_sb = [qkvp.tile([P, FW], bf16, name=f"v{i}")
                            for i in range(TOK // P)]

                    # ================= PHASE A: ln1 + qkv =================
                    with (
                        tc.tile_pool(name="wqkv", bufs=1) as wpool,
                        tc.tile_pool(name="xchunk", bufs=2) as xpool,
                        tc.tile_pool(name="arows", bufs=2) as rows,
                        tc.tile_pool(name="astage", bufs=1) as stg,
                        tc.tile_pool(name="ps_st", bufs=1, space="PSUM") as pst,
                        tc.tile_pool(name="ps_bc", bufs=1, space="PSUM") as pbc,
                        tc.tile_pool(name="ps_qk", bufs=3, space="PSUM") as pqk,
                        tc.tile_pool(name="ps_v", bufs=2, space="PSUM") as pv,
                    ):
                        # chunk-0 x first (split) so stats matmuls start ASAP
                        xb0 = xpool.tile([P, KT * 512], bf16, tag="xb",
                                         name="xb0")
                        for q4 in range(4):
                            nc.sync.dma_start(
                                out=xb0[:, q4 * 2048:(q4 + 1) * 2048],
                                in_=xTt[0, :, q4 * 2048:(q4 + 1) * 2048])
                        wq_s = wpool.tile([P, KT * FW], bf16, tag="wq")
                        wk_s = wpool.tile([P, KT * FW], bf16, tag="wk")
                        wv_s = wpool.tile([P, KT * FW], bf16, tag="wv")
                        nc.sync.dma_start(out=wq_s[:], in_=wq[:, :])
                        nc.sync.dma_start(out=wk_s[:], in_=wk[:, :])
                        nc.sync.dma_start(out=wv_s[:], in_=wv[:, :])
                        # remaining constants (not needed by the first mms)
                        ones_sqb = const.tile([P, P], bf16)     # denom lhsT
                        nc.sync.dma_start(out=ones_sqb[:], in_=ones_b[:, :])
                        masks = []
                        for d in range(2):
                            m = const.tile([P, 1024], bf16, name=f"mask{d}")
                            nc.sync.dma_start(
                                out=m[:], in_=masks_in[d * P:(d + 1) * P, :])
                            masks.append(m)
                        bpjc_s = const.tile([P, KT], f32)
                        nc.sync.dma_start(out=bpjc_s[:], in_=bpjc[:, :])
                        bfcc_s = const.tile([P, FF // P], f32)
                        nc.sync.dma_start(out=bfcc_s[:], in_=bfcc[:, :])
                        bf2c_s = const.tile([P, KT], f32)
                        nc.sync.dma_start(out=bf2c_s[:], in_=bf2c[:, :])
                        csq_s = const.tile([1, 3 * FW], bf16)
                        nc.sync.dma_start(out=csq_s[:], in_=csqkv[:, :])
                        if gb1:
                            b_rows = {}
                            for nm, src in (("q", bqr), ("k", bkr),
                                            ("v", bvr)):
                                t = const.tile([1, FW], bf16,
                                               name=f"brow_{nm}")
                                nc.sync.dma_start(out=t[:], in_=src[:, :])
                                b_rows[nm] = t

                        for c in range(NCH):
                            tok0 = c * 512
                            if c == 0:
                                xb = xb0
                            else:
                                xb = xpool.tile([P, KT * 512], bf16,
                                                tag="xb", name=f"xb{c}")
                                nc.sync.dma_start(out=xb[:], in_=xTt[c, :, :])
                            xk = [xb[:, k * 512:(k + 1) * 512]
                                  for k in range(KT)]
                            # --- stats: 4-way DVE pre-reduction over k-tiles
                            # keeps only 4+4 stats matmuls per chunk on PE
                            stx = pst.tile([1, 512], f32, tag="stx")
                            stq = pst.tile([1, 512], f32, tag="stq")
                            for q4 in range(4):
                                ks = [xk[4 * q4 + i] for i in range(4)]
                                sqs = []
                                for i in range(4):
                                    sq = stg.tile([P, 512], bf16, tag="sq",
                                                  bufs=5, name=f"sq{q4}_{i}")
                                    nc.vector.tensor_tensor(
                                        sq[:], ks[i], ks[i], ALU.mult)
                                    sqs.append(sq)
                                xp0 = stg.tile([P, 512], bf16, tag="xp",
                                               bufs=4, name=f"xp0_{q4}")
                                xp1 = stg.tile([P, 512], bf16, tag="xp",
                                               bufs=4, name=f"xp1_{q4}")
                                xqd = stg.tile([P, 512], bf16, tag="xqd",
                                               bufs=2, name=f"xqd{q4}")
                                nc.vector.tensor_tensor(xp0[:], ks[0], ks[1],
                                                        ALU.add)
                                nc.vector.tensor_tensor(xp1[:], ks[2], ks[3],
                                                        ALU.add)
                                nc.vector.tensor_tensor(xqd[:], xp0[:],
                                                        xp1[:], ALU.add)
                                sp0 = stg.tile([P, 512], bf16, tag="xp",
                                               bufs=4, name=f"sp0_{q4}")
                                sp1 = stg.tile([P, 512], bf16, tag="xp",
                                               bufs=4, name=f"sp1_{q4}")
                                sqd = stg.tile([P, 512], bf16, tag="sqd",
                                               bufs=2, name=f"sqd{q4}")
                                nc.vector.tensor_tensor(sp0[:], sqs[0][:],
                                                        sqs[1][:], ALU.add)
                                nc.vector.tensor_tensor(sp1[:], sqs[2][:],
                                                        sqs[3][:], ALU.add)
                                nc.vector.tensor_tensor(sqd[:], sp0[:],
                                                        sp1[:], ALU.add)
                                nc.tensor.matmul(stx[:], ones_colb[:],
                                                 xqd[:], start=(q4 == 0),
                                                 stop=(q4 == 3))
                                nc.tensor.matmul(stq[:], ones_colb[:],
                                                 sqd[:], start=(q4 == 0),
                                                 stop=(q4 == 3))
                            negmu = rows.tile([1, 512], f32r, tag="negmu")
                            negmuh = rows.tile([1, 512], bf16, tag="negmuh")
                            ex2 = rows.tile([1, 512], f32, tag="ex2")
                            mu2 = rows.tile([1, 512], f32, tag="mu2")
                            var = rows.tile([1, 512], f32, tag="var")
                            std = rows.tile([1, 512], f32r, tag="std")
                            rrow = rows.tile([1, 512], f32r, tag="rrow")
                            if gb1:
                                stdh = rows.tile([1, 512], bf16, tag="stdh")
                            nc.vector.tensor_scalar_mul(negmu[:], stx[:],
                                                        -1.0 / C)
                            nc.vector.tensor_copy(negmuh[:], negmu[:])
                            nc.vector.tensor_scalar_mul(ex2[:], stq[:],
                                                        1.0 / C)
                            nc.vector.tensor_tensor(mu2[:], negmu[:],
                                                    negmu[:], ALU.mult)
                            nc.vector.tensor_tensor(var[:], ex2[:], mu2[:],
                                                    ALU.subtract)
                            nc.scalar.activation(std[:], var[:], AF.Sqrt,
                                                 bias=eps_col[0:1, :])
                            nc.vector.reciprocal(rrow[:], std[:])
                            if gb1:
                                nc.vector.tensor_copy(stdh[:], std[:])
                            # r as column form (4 outer products) + bcast
                            rcolp = pbc.tile([P, 4], f32, tag="rbp",
                                             name=f"rcolp{c}")
                            for m in range(4):
                                nc.tensor.matmul(
                                    rcolp[:, m:m + 1],
                                    rrow[0:1, m * P:(m + 1) * P].bitcast(f32),
                                    ones_row[0:1, 0:1].bitcast(f32),
                                    start=True, stop=True)
                            rcol = rows.tile([P, 4], f32, tag="rcol")
                            nc.scalar.copy(rcol[:], rcolp[:])
                            rbp = pbc.tile([P, 512], f32, tag="rbp")
                            nc.tensor.matmul(rbp[:], rr(ones_row[:]),
                                             rr(rrow[:]), start=True,
                                             stop=True)
                            rb_s = stg.tile([P, 512], f32, tag="rb", bufs=2)
                            nc.scalar.copy(rb_s[:], rbp[:])

                            # Q^T, K^T (feat x tok) on raw x + rank-1 fix
                            for qk_i, (ws, dst) in enumerate(
                                    ((wq_s, qT_sb), (wk_s, kT_sb))):
                                for m in range(HPC):
                                    pq = pqk.tile([P, 512], f32, tag="pqk")
                                    for k in range(KT):
                                        nc.tensor.matmul(
                                            pq[:],
                                            ws[:, k * FW + m * P:
                                               k * FW + (m + 1) * P],
                                            xk[k],
                                            start=(k == 0), stop=False)
                                    cs0 = qk_i * FW + m * P
                                    nc.tensor.matmul(
                                        pq[:], csq_s[0:1, cs0:cs0 + P],
                                        negmuh[:], start=False,
                                        stop=(not gb1))
                                    if gb1:
                                        br = b_rows["q" if qk_i == 0 else "k"]
                                        nc.tensor.matmul(
                                            pq[:], br[0:1, m * P:(m + 1) * P],
                                            stdh[:], start=False, stop=True)
                                    nc.vector.tensor_tensor(
                                        dst[m][:, tok0:tok0 + 512],
                                        pq[:], rb_s[:], ALU.mult)
                            # V (tok x feat) on raw x + rank-1 fix
                            for mt in range(4):
                                pvt = pv.tile([P, FW], f32, tag="pv")
                                for k in range(KT):
                                    nc.tensor.matmul(
                                        pvt[:],
                                        xk[k][:, mt * P:(mt + 1) * P],
                                        wv_s[:, k * FW:(k + 1) * FW],
                                        start=(k == 0), stop=False)
                                nc.tensor.matmul(
                                    pvt[:],
                                    negmuh[0:1, mt * P:(mt + 1) * P],
                                    csq_s[0:1, 2 * FW:3 * FW],
                                    start=False, stop=(not gb1))
                                if gb1:
                                    nc.tensor.matmul(
                                        pvt[:],
                                        stdh[0:1, mt * P:(mt + 1) * P],
                                        b_rows["v"][:],
                                        start=False, stop=True)
                                nc.scalar.activation(
                                    v_sb[c * 4 + mt][:], pvt[:], AF.Copy,
                                    scale=rcol[:, mt:mt + 1])

                    # ================= PHASE B: attention =================
                    with (
                        tc.tile_pool(name="expp", bufs=3) as ep,
                        tc.tile_pool(name="bstage", bufs=2) as bstg,
                        tc.tile_pool(name="ystage", bufs=2) as ystg,
                        tc.tile_pool(name="ps_sc", bufs=2, space="PSUM") as psc,
                        tc.tile_pool(name="ps_dn", bufs=2, space="PSUM") as pdn,
                        tc.tile_pool(name="ps_y", bufs=2, space="PSUM") as psy,
                    ):
                        for u, (hl, bb) in enumerate(
                                ((0, 0), (0, 1), (1, 0), (1, 1))):
                            qhb = qT_sb[hl][:, bb * T:(bb + 1) * T]
                            khb = kT_sb[hl][:, bb * T:(bb + 1) * T]
                            yT = ystg.tile([P, T], bf16, tag="yT",
                                           name=f"yT{u}")
                            for qc in range(T // 512):
                                nk = 4 * (qc + 1)
                                ebigs = []
                                for g in range(nk // 2):
                                    ps = psc.tile([P, 1024], f32, tag="sc")
                                    for i in range(2):
                                        kt = 2 * g + i
                                        nc.tensor.matmul(
                                            ps[:, i * 512:(i + 1) * 512],
                                            khb[:, kt * P:(kt + 1) * P],
                                            qhb[:, qc * 512:(qc + 1) * 512],
                                            start=True, stop=True)
                                    e = ep.tile([P, 1024], bf16, tag="e",
                                                name=f"e{g}", bufs=8)
                                    if 2 * g >= 4 * qc:
                                        etmp = bstg.tile([P, 1024], bf16,
                                                         tag="ed", bufs=3)
                                        nc.scalar.activation(etmp[:], ps[:],
                                                             AF.Exp,
                                                             scale=ISQ)
                                        nc.vector.tensor_tensor(
                                            e[:], etmp[:],
                                            masks[(2 * g - 4 * qc) // 2][:],
                                            ALU.mult)
                                    else:
                                        nc.scalar.activation(e[:], ps[:],
                                                             AF.Exp,
                                                             scale=ISQ)
                                    ebigs.append(e)
                                pd = pdn.tile([P, 512], f32, tag="pd")
                                py = psy.tile([P, 512], f32, tag="py")
                                for kt in range(nk):
                                    sl = ebigs[kt // 2][:, (kt % 2) * 512:
                                                        (kt % 2) * 512 + 512]
                                    nc.tensor.matmul(pd[:], ones_sqb[:], sl,
                                                     start=(kt == 0),
                                                     stop=(kt == nk - 1))
                                    vt = v_sb[bb * 16 + kt]
                                    nc.tensor.matmul(
                                        py[:], vt[:, hl * P:(hl + 1) * P], sl,
                                        start=(kt == 0), stop=(kt == nk - 1))
                                rc = bstg.tile([P, 512], f32, tag="rc",
                                               bufs=2)
                                nc.vector.reciprocal(rc[:], pd[:])
                                nc.vector.tensor_tensor(
                                    yT[:, qc * 512:(qc + 1) * 512],
                                    py[:], rc[:], ALU.mult)
                            for j in range(4):
                                nc.sync.dma_start(
                                    out=a2a_in[hl][(bb * 4 + j) * P:
                                                   (bb * 4 + j + 1) * P, :],
                                    in_=yT[:, j * 512:(j + 1) * 512])
                            if bb == 1:
                                if n_cores > 1:
                                    nc.gpsimd.collective_compute(
                                        "AllToAll", ALU.bypass,
                                        replica_groups=[list(range(n_cores))],
                                        ins=[a2a_in[hl][:, :].opt()],
                                        outs=[a2a_out[hl][:, :].opt()],
                                    )
                                else:
                                    nc.sync.dma_start(out=a2a_out[hl][:, :],
                                                      in_=a2a_in[hl][:, :])

                # =============== PHASE C: proj (own tokens) ===============
                with (
                    tc.tile_pool(name="x2pool", bufs=1) as x2p,
                    tc.tile_pool(name="drows", bufs=1) as drows,
                ):
                    acc = [x2p.tile([P, TPC], f32, name=f"acc{m}")
                           for m in range(KT)]
                    x2b = [x2p.tile([P, TPC], bf16, name=f"x2b{m}")
                           for m in range(KT)]
                    negmu2 = drows.tile([1, TPC], f32r, tag="negmu2")
                    negmu2h = drows.tile([1, TPC], bf16, tag="negmu2h")
                    r2b_s = drows.tile([P, TPC], f32, tag="r2b")
                    with (
                        tc.tile_pool(name="wpj_p", bufs=1) as wpp,
                        tc.tile_pool(name="ygp", bufs=1) as ygp,
                        tc.tile_pool(name="cstage", bufs=2) as cstg,
                        tc.tile_pool(name="ps_pj", bufs=3, space="PSUM") as ppj,
                        tc.tile_pool(name="ps_st2", bufs=1,
                                     space="PSUM") as pst2,
                        tc.tile_pool(name="ps_bc2", bufs=1,
                                     space="PSUM") as pbc2,
                    ):
                        st2x = pst2.tile([1, TPC], f32, tag="st2x")
                        st2q = pst2.tile([1, TPC], f32, tag="st2q")
                        wpj_s = {}
                        yg = {}

                        def _load_pj(hl):
                            for j in range(NCH):
                                w = wpp.tile([P, C], bf16, tag=f"wpj{hl}_{j}",
                                             name=f"wpj{hl}_{j}")
                                nc.sync.dma_start(out=w[:],
                                                  in_=wpj[hl * NCH + j, :, :])
                                wpj_s[(hl, j)] = w
                            for j in range(NCH):
                                y = ygp.tile([P, 512], bf16,
                                             tag=f"yg{hl}_{j}",
                                             name=f"yg{hl}_{j}")
                                nc.sync.dma_start(
                                    out=y[:],
                                    in_=a2a_out[hl][j * P:(j + 1) * P, :])
                                yg[(hl, j)] = y

                        _load_pj(0)
                        for hl in range(2):
                            if hl == 1:
                                # deferred: keeps the pass-0 xmy loads ahead
                                # of the A2A_1-gated reads in the DMA FIFO
                                _load_pj(1)
                            for m in range(KT):
                                pp = ppj.tile([P, TPC], f32, tag="pp")
                                for j in range(NCH):
                                    nc.tensor.matmul(
                                        pp[:],
                                        wpj_s[(hl, j)][:, m * P:(m + 1) * P],
                                        yg[(hl, j)][:],
                                        start=(j == 0), stop=(j == NCH - 1))
                                if hl == 0:
                                    xmy = cstg.tile([P, TPC], f32, tag="xmy",
                                                    bufs=3)
                                    nc.sync.dma_start(
                                        out=xmy[:],
                                        in_=xTm[m * P:(m + 1) * P, :])
                                    nc.vector.scalar_tensor_tensor(
                                        acc[m][:], pp[:], bpjc_s[:, m:m + 1],
                                        xmy[:], ALU.add, ALU.add)
                                else:
                                    nc.vector.tensor_tensor(acc[m][:], pp[:],
                                                            acc[m][:],
                                                            ALU.add)
                                    nc.vector.tensor_copy(x2b[m][:],
                                                          acc[m][:])
                                    sq2 = cstg.tile([P, TPC], bf16,
                                                    tag="sq2", bufs=3)
                                    nc.vector.tensor_tensor(sq2[:],
                                                            x2b[m][:],
                                                            x2b[m][:],
                                                            ALU.mult)
                                    nc.tensor.matmul(st2x[:], ones_colb[:],
                                                     x2b[m][:],
                                                     start=(m == 0),
                                                     stop=(m == KT - 1))
                                    nc.tensor.matmul(st2q[:], ones_colb[:],
                                                     sq2[:], start=(m == 0),
                                                     stop=(m == KT - 1))
                        # ln2 row stats
                        ex22 = drows.tile([1, TPC], f32, tag="ex22")
                        mu22 = drows.tile([1, TPC], f32, tag="mu22")
                        var2 = drows.tile([1, TPC], f32, tag="var2")
                        std2 = drows.tile([1, TPC], f32r, tag="std2")
                        rrow2 = drows.tile([1, TPC], f32r, tag="rrow2")
                        nc.vector.tensor_scalar_mul(negmu2[:], st2x[:],
                                                    -1.0 / C)
                        nc.vector.tensor_copy(negmu2h[:], negmu2[:])
                        nc.vector.tensor_scalar_mul(ex22[:], st2q[:],
                                                    1.0 / C)
                        nc.vector.tensor_tensor(mu22[:], negmu2[:],
                                                negmu2[:], ALU.mult)
                        nc.vector.tensor_tensor(var2[:], ex22[:], mu22[:],
                                                ALU.subtract)
                        nc.scalar.activation(std2[:], var2[:], AF.Sqrt,
                                             bias=eps_col[0:1, :])
                        nc.vector.reciprocal(rrow2[:], std2[:])
                        rb2p = pbc2.tile([P, TPC], f32, tag="bc2")
                        nc.tensor.matmul(rb2p[:], rr(ones_row[:]),
                                         rr(rrow2[:]), start=True, stop=True)
                        nc.scalar.copy(r2b_s[:], rb2p[:])

                    # ===================== PHASE D: MLP =====================
                    with (
                        tc.tile_pool(name="wfpool", bufs=3) as wfp,
                        tc.tile_pool(name="wgpool", bufs=3) as wgp,
                        tc.tile_pool(name="apool", bufs=1) as apool,
                        tc.tile_pool(name="dstage", bufs=3) as dstg,
                        tc.tile_pool(name="ps_f", bufs=2, space="PSUM") as pf,
                        tc.tile_pool(name="ps_g", bufs=3, space="PSUM") as pg,
                    ):
                        csfc_s = drows.tile([1, FF], bf16, tag="csfc")
                        nc.sync.dma_start(out=csfc_s[:], in_=csfc[:, :])
                        for ch in range(NCH):
                            aT = [apool.tile([P, TPC], bf16, tag=f"aT{m}",
                                             name=f"aT{ch}_{m}", bufs=2)
                                  for m in range(8)]
                            for m in range(8):
                                wfm = wfp.tile([P, KT * P], bf16, tag="wfm",
                                               name=f"wfm{ch}_{m}")
                                nc.sync.dma_start(out=wfm[:],
                                                  in_=wfc[ch * 8 + m, :, :])
                                pft = pf.tile([P, TPC], f32, tag="pf")
                                for k in range(KT):
                                    nc.tensor.matmul(
                                        pft[:], wfm[:, k * P:(k + 1) * P],
                                        x2b[k][:], start=(k == 0),
                                        stop=False)
                                f0 = (ch * 8 + m) * P
                                nc.tensor.matmul(
                                    pft[:], csfc_s[0:1, f0:f0 + P],
                                    negmu2h[:], start=False, stop=True)
                                tmp = dstg.tile([P, TPC], f32, tag="tmp",
                                                bufs=3)
                                nc.vector.tensor_tensor(tmp[:], pft[:],
                                                        r2b_s[:], ALU.mult)
                                nc.scalar.activation(
                                    aT[m][:], tmp[:], AF.Gelu,
                                    bias=bfcc_s[:, ch * 8 + m:ch * 8 + m + 1])
                            for m in range(KT):
                                wgm = wgp.tile([P, 8 * P], bf16, tag="wgm",
                                               name=f"wgm{ch}_{m}")
                                nc.sync.dma_start(out=wgm[:],
                                                  in_=wfc2[ch, m, :, :])
                                pgt = pg.tile([P, TPC], f32, tag="pg")
                                for kk in range(8):
                                    nc.tensor.matmul(
                                        pgt[:], wgm[:, kk * P:(kk + 1) * P],
                                        aT[kk][:], start=(kk == 0),
                                        stop=(kk == 7))
                                if ch == 0:
                                    nc.vector.scalar_tensor_tensor(
                                        acc[m][:], pgt[:],
                                        bf2c_s[:, m:m + 1], acc[m][:],
                                        ALU.add, ALU.add)
                                else:
                                    nc.vector.tensor_tensor(
                                        acc[m][:], pgt[:], acc[m][:],
                                        ALU.add)
                                if ch == NCH - 1:
                                    nc.sync.dma_start(
                                        out=out[m * P:(m + 1) * P, :],
                                        in_=acc[m][:])

    nc.compile()
    return nc


def _get_program(n_cores, gb1):
    key = (n_cores, gb1)
    if key not in _BUILD_CACHE:
        _BUILD_CACHE[key] = _build_program(n_cores, gb1)
    return _BUILD_CACHE[key]


def _colmajor(v, kt):
    """(kt*128,) vector -> (128, kt) column-tile layout."""
    return np.ascontiguousarray(v.reshape(kt, P).T)


def make_in_maps(x, ln1_w, ln1_b, w_qkv, b_qkv, w_proj, b_proj,
                 ln2_w, ln2_b, w_fc, b_fc, w_fc2, b_fc2, n_cores=N_CORES):
    """Host-side sharding: slicing / transpose / fold / reshape only."""
    f = np.float32
    bf = np.dtype("bfloat16") if hasattr(np, "bfloat16") else None
    import ml_dtypes
    bf = ml_dtypes.bfloat16
    x2d = np.ascontiguousarray(np.asarray(x, f).reshape(TOK, C))
    xT = np.ascontiguousarray(x2d.T)

    # fold ln weights into the projection weights (host-side)
    w_qkv_e = np.asarray(ln1_w, f)[:, None] * np.asarray(w_qkv, f)
    w_fc_e = np.asarray(ln2_w, f)[:, None] * np.asarray(w_fc, f)
    bq_e = np.asarray(ln1_b, f) @ w_qkv_e + np.asarray(b_qkv, f)
    bfc_e = np.asarray(ln2_b, f) @ w_fc_e + np.asarray(b_fc, f)
    gb1 = bool(np.any(bq_e != 0.0))

    # causal mask pair-tiles: mask[d][kk, i*512+qq] = 1 if qq - kk - 128*(2d+i) >= 0
    _kk = np.arange(P)[:, None]
    _qq = np.arange(512)[None, :]
    _m4 = [(_qq - _kk - P * d >= 0).astype(f) for d in range(4)]
    _masks = np.concatenate(
        [np.concatenate([_m4[2 * d], _m4[2 * d + 1]], axis=1)
         for d in range(2)], axis=0).astype(bf)

    wfc_t = np.ascontiguousarray(
        w_fc_e.reshape(KT, P, FF // P, P)
        .transpose(2, 1, 0, 3).reshape(FF // P, P, KT * P)).astype(bf)
    wfc2_t = np.ascontiguousarray(
        np.asarray(w_fc2, f).reshape(8, 8, P, KT, P)
        .transpose(0, 3, 2, 1, 4).reshape(8, KT, P, 8 * P)).astype(bf)
    # x, transposed, chunk-major [NCH, P, KT*512]
    xT_t = np.ascontiguousarray(
        xT.reshape(KT, P, NCH, 512).transpose(2, 1, 0, 3)
        .reshape(NCH, P, KT * 512)).astype(bf)
    # w_proj rows grouped by (hl, j): block hl*8+j = head (2j+hl)
    wp = np.asarray(w_proj, f).reshape(H, P, C)
    wpj_t = np.ascontiguousarray(
        np.stack([wp[2 * j + hl] for hl in range(2) for j in range(NCH)],
                 axis=0)).astype(bf)
    csfc_r = w_fc_e.sum(axis=0)[None, :].astype(bf)

    shared = {
        "xTt": xT_t,
        "ones_f": np.ones((P, P), f),
        "ones_b": np.ones((P, P), bf),
        "masks_in": _masks,
        "wpj": wpj_t,
        "wfc": wfc_t,
        "csfc": csfc_r,
        "wfc2": wfc2_t,
        "bpjc": _colmajor(np.asarray(b_proj, f), KT),
        "bfcc": _colmajor(bfc_e, FF // P),
        "bf2c": _colmajor(np.asarray(b_fc2, f), KT),
    }
    in_maps = []
    for c in range(n_cores):
        m = dict(shared)
        m["xTm"] = np.ascontiguousarray(xT[:, c * TPC:(c + 1) * TPC])

        def _kpf(w):  # (C, FW) -> (P p, KT*FW kf)
            return np.ascontiguousarray(
                w.reshape(KT, P, FW).transpose(1, 0, 2).reshape(P, KT * FW))
        wqc = w_qkv_e[:, c * FW:(c + 1) * FW]
        wkc = w_qkv_e[:, C + c * FW:C + (c + 1) * FW]
        wvc = w_qkv_e[:, 2 * C + c * FW:2 * C + (c + 1) * FW]
        m["wq"] = _kpf(wqc).astype(bf)
        m["wk"] = _kpf(wkc).astype(bf)
        m["wv"] = _kpf(wvc).astype(bf)
        m["csqkv"] = np.concatenate(
            [wqc.sum(axis=0), wkc.sum(axis=0), wvc.sum(axis=0)])[None, :]\
            .astype(bf)
        if gb1:
            m["bqr"] = bq_e[None, c * FW:(c + 1) * FW].astype(bf)
            m["bkr"] = bq_e[None, C + c * FW:C + (c + 1) * FW].astype(bf)
            m["bvr"] = bq_e[None, 2 * C + c * FW:2 * C + (c + 1) * FW].astype(bf)
        in_maps.append(m)
    return in_maps, gb1


def kernel(**inputs):
    from concourse.bass_utils import run_bass_kernel_spmd

    in_maps, gb1 = make_in_maps(**inputs)
    nc = _get_program(N_CORES, gb1)

    trace = os.environ.get("KERNEL_TRACE", "0") == "1"
    kw = {}
    if trace:
        kw = dict(trace=True)
    try:
        res = run_bass_kernel_spmd(nc, in_maps, list(range(N_CORES)), **kw)
    except Exception as e:
        if not trace:
            raise
        _LAST_RESULTS["trace_error"] = repr(e)
        res = run_bass_kernel_spmd(nc, in_maps, list(range(N_CORES)))
    _LAST_RESULTS["exec_time_ns"] = res.exec_time_ns
    _LAST_RESULTS["mean_exec_time_ns"] = res.mean_exec_time_ns
    _LAST_RESULTS["results"] = res
    outT = np.concatenate([res.results[i]["out"] for i in range(N_CORES)],
                          axis=1)
    return np.ascontiguousarray(outT.T).reshape(B, T, C).astype(np.float32)
